# revision 1
# baseline (speedup 1.0000x reference)
"""Trainium2 8-core kernel for nn_EnhancedTransformerBlock.

SPMD: identical program on all 8 cores, only in_maps data differs.
  - Sequence-sharded everywhere except attention: core c owns tokens
    [256c, 256c+256), activations in T-layout [feature, token].
  - Attention head-sharded (2 of 16 heads per core, full sequence).
    AllGather of ln(x) (bf16) before QKV; AllToAll of per-head attention
    outputs back to sequence sharding.
  - All GEMMs bf16 (weights pre-transposed/packed host-side), fp32 PSUM.
  - Softmax: temperature (1/0.1) and 1/sqrt(hd) folded into Wq; unshifted
    exp (max |logit| ~35 fits fp32); denominator via appended ones-column
    on V; causal masking via triangle-mask multiply on diagonal blocks.
  - Spline activation g(u) is a fixed 1-D function of u determined by
    knots/spl_w; approximated by a kink-basis least-squares fit computed
    on the host from the runtime knots/spl_w (max err ~6e-4 on |u|<=0.2;
    |u| = |lnh|/(norm+1) < 0.1 in practice). The e-gate path is computed
    exactly on device.
"""

import hashlib
import numpy as np

from concourse import bacc, tile, mybir
from concourse import bass_utils

dt = mybir.dt
BF = dt.bfloat16
F32 = dt.float32
NPBF = dt.np(BF)
Alu = mybir.AluOpType
Act = mybir.ActivationFunctionType

NCORES = 8
S = 2048
D = 1024
H = 16
HD = 64
FF = 4096
D16 = 256
TOK = S // NCORES            # 256 tokens per core
HPC = H // NCORES            # 2 heads per core
EPS = 1e-6
UDOM = 0.2                   # spline fit domain |u| <= UDOM

_prog_cache = {}


# ----------------------------------------------------------------------------
# Host-side: spline fit
# ----------------------------------------------------------------------------

def _g_exact(u, knots, spl_w):
    d = np.abs(u[:, None] - knots[None, :])
    d = d / (d.max(-1, keepdims=True) + EPS)
    a = -5.0 * d
    a = a - a.max(-1, keepdims=True)
    e = np.exp(a)
    p = e / e.sum(-1, keepdims=True)
    return (p * spl_w).sum(-1)


def _fit_spline(knots, spl_w):
    """LSQ fit of g(u) on [-UDOM, UDOM]; basis
    [1, u, u^2, |u|, |u|u, u^3, |u-b|, |u+b|]. Returns (coeffs, b, maxerr)."""
    k = np.asarray(knots, np.float64)
    w = np.asarray(spl_w, np.float64)
    pos = np.sort(np.abs(k[(np.abs(k) > 1e-9) & (np.abs(k) < UDOM * 0.9)]))
    b = float(pos[0]) if len(pos) else UDOM / 2.0
    u = np.linspace(-UDOM, UDOM, 20001)
    B = np.stack([np.ones_like(u), u, u * u, np.abs(u),
                  np.abs(u - b), np.abs(u + b)], -1)
    y = _g_exact(u, k, w)
    c, *_ = np.linalg.lstsq(B, y, rcond=None)
    err = float(np.abs(B @ c - y).max())
    cc = [float(c[0]), float(c[1]), float(c[2]), float(c[3]), 0.0, 0.0,
          float(c[4]), float(c[5])]
    return cc, b, err


# ----------------------------------------------------------------------------
# Host-side: weight packing
# ----------------------------------------------------------------------------

def _pack_lhsT(w_t, n_of, n_kc):
    """w_t: [K_total, M_total] ([in, out]) -> [128, n_of*n_kc*128], tile
    (of, kc) at cols [(of*n_kc+kc)*128 ...] = w_t[128kc:.., 128of:..]."""
    K_total, M_total = w_t.shape
    assert K_total == n_kc * 128 and M_total == n_of * 128
    out = np.empty((128, n_of * n_kc * 128), np.float32)
    for of in range(n_of):
        for kc in range(n_kc):
            out[:, (of * n_kc + kc) * 128:(of * n_kc + kc + 1) * 128] = \
                w_t[kc * 128:(kc + 1) * 128, of * 128:(of + 1) * 128]
    return np.ascontiguousarray(out)


def _col_pack(vec, n_chunks):
    return np.ascontiguousarray(
        np.asarray(vec, np.float32).reshape(n_chunks, 128).T)


def _make_tri_masks():
    out = np.zeros((128, 4 * 512), np.float32)
    for j in range(4):
        kk = np.arange(128)[:, None] + 128 * j
        q = np.arange(512)[None, :]
        out[:, 512 * j:512 * (j + 1)] = (kk <= q).astype(np.float32)
    return out


def _prepare_inputs(inputs):
    f = lambda k: np.asarray(inputs[k], np.float32)
    x = f("x").reshape(S, D)
    qkv_w, qkv_b = f("qkv_w"), f("qkv_b")
    out_w, out_b = f("out_w") * 0.1, f("out_b") * 0.1
    ff1_w, ff1_b = f("ff1_w"), f("ff1_b")
    ff2_w, ff2_b = f("ff2_w"), f("ff2_b")
    ep1_w, ep1_b = f("ep1_w"), f("ep1_b")
    ep2_w, ep2_b = f("ep2_w"), f("ep2_b")
    ent_w, ent_b = f("ent_w"), f("ent_b")

    temp = (1.0 / np.sqrt(np.float32(HD))) / 0.1   # 1.25
    wq = qkv_w[0:D] * temp
    wk = qkv_w[D:2 * D]
    wv = qkv_w[2 * D:3 * D]
    bq = qkv_b[0:D] * temp
    bk = qkv_b[D:2 * D]
    bv = qkv_b[2 * D:3 * D]

    coeffs, bknot, fit_err = _fit_spline(f("knots"), f("spl_w"))

    shared = {
        "tri": _make_tri_masks().astype(NPBF),
        "ones32": np.ones((128, 1), np.float32),
        "onesb": np.ones((128, 1), NPBF),
        "wff1": _pack_lhsT(ff1_w.T, 32, 8).astype(NPBF),
        "wff2": _pack_lhsT(ff2_w.T, 8, 32).astype(NPBF),
        "wep1": _pack_lhsT(ep1_w.T, 2, 32).astype(NPBF),
        "wout": _pack_lhsT(out_w.T, 8, 8).astype(NPBF),
        "wep2": np.ascontiguousarray(
            ep2_w.reshape(2, 128).T).astype(NPBF),          # [128, 2]
        "went": np.ascontiguousarray(
            ent_w.reshape(8, 128).T).astype(NPBF),          # [128, 8]
        "b_ff1": _col_pack(ff1_b, 32),
        "b_ff2": _col_pack(ff2_b, 8),
        "b_ep1": _col_pack(ep1_b, 2),
        "b_out": _col_pack(out_b, 8),
        "lnw": _col_pack(f("ln_attn_w"), 8),
        "lnb": _col_pack(f("ln_attn_b"), 8),
        "n1w": _col_pack(f("norm1_w"), 8),
        "n1b": _col_pack(f("norm1_b"), 8),
        "n2w": _col_pack(f("norm2_w"), 8),
        "n2b": _col_pack(f("norm2_b"), 8),
        "eplw": _col_pack(f("ep_ln_w"), 2),
        "eplb": _col_pack(f("ep_ln_b"), 2),
    }

    scalars = {
        "ent_b": float(ent_b.reshape(-1)[0]),
        "ep2_b": float(ep2_b.reshape(-1)[0]),
        "coeffs": coeffs,
        "bknot": bknot,
        "fit_err": fit_err,
    }

    in_maps = []
    for c in range(NCORES):
        m = dict(shared)
        xc = x[c * TOK:(c + 1) * TOK]                        # [256, D]
        xT = np.ascontiguousarray(xc.T)                      # [D, 256]
        m["xT"] = np.ascontiguousarray(
            xT.reshape(8, 128, TOK).transpose(1, 0, 2).reshape(128, 8 * TOK))
        h0 = c * HPC
        wq_c = wq[h0 * HD:(h0 + HPC) * HD]                   # [128, D]
        wk_c = wk[h0 * HD:(h0 + HPC) * HD]
        wqk_t = np.concatenate([wq_c, wk_c], 0).T            # [D, 256]
        m["wqk"] = _pack_lhsT(wqk_t, 2, 8).astype(NPBF)
        m["b_qk"] = np.ascontiguousarray(np.stack(
            [bq[h0 * HD:(h0 + HPC) * HD],
             bk[h0 * HD:(h0 + HPC) * HD]], -1).astype(np.float32))
        wv_c = wv[h0 * HD:(h0 + HPC) * HD].T                 # [D, 128]
        wva = np.zeros((D, 136), np.float32)
        bva = np.zeros((1, 136), np.float32)
        for lh in range(HPC):
            wva[:, 68 * lh:68 * lh + 64] = wv_c[:, 64 * lh:64 * lh + 64]
            bva[0, 68 * lh:68 * lh + 64] = \
                bv[(h0 + lh) * HD:(h0 + lh + 1) * HD]
        m["wv"] = np.ascontiguousarray(
            wva.reshape(8, 128, 136).transpose(1, 0, 2).reshape(128, 8 * 136)
        ).astype(NPBF)
        m["b_v"] = bva
        in_maps.append(m)

    return in_maps, scalars


# ----------------------------------------------------------------------------
# Device program
# ----------------------------------------------------------------------------

def _build_program(sc):
    nc = bacc.Bacc("TRN2", target_bir_lowering=False, debug=False,
                   num_devices=NCORES)

    def din(name, shape, dtype):
        return nc.dram_tensor(name, list(shape), dtype, kind="ExternalInput")

    tin = {
        "xT": din("xT", (128, 8 * TOK), F32),
        "wqk": din("wqk", (128, 2048), BF),
        "wv": din("wv", (128, 8 * 136), BF),
        "went": din("went", (128, 8), BF),
        "wout": din("wout", (128, 8192), BF),
        "wff1": din("wff1", (128, 32768), BF),
        "wff2": din("wff2", (128, 32768), BF),
        "wep1": din("wep1", (128, 8192), BF),
        "wep2": din("wep2", (128, 2), BF),
        "tri": din("tri", (128, 2048), BF),
        "ones32": din("ones32", (128, 1), F32),
        "onesb": din("onesb", (128, 1), BF),
        "b_qk": din("b_qk", (128, 2), F32),
        "b_v": din("b_v", (1, 136), F32),
        "b_out": din("b_out", (128, 8), F32),
        "b_ff1": din("b_ff1", (128, 32), F32),
        "b_ff2": din("b_ff2", (128, 8), F32),
        "b_ep1": din("b_ep1", (128, 2), F32),
        "lnw": din("lnw", (128, 8), F32),
        "lnb": din("lnb", (128, 8), F32),
        "n1w": din("n1w", (128, 8), F32),
        "n1b": din("n1b", (128, 8), F32),
        "n2w": din("n2w", (128, 8), F32),
        "n2b": din("n2b", (128, 8), F32),
        "eplw": din("eplw", (128, 2), F32),
        "eplb": din("eplb", (128, 2), F32),
    }
    t_out = nc.dram_tensor("out", [128, 8 * TOK], F32, kind="ExternalOutput")
    import os
    dbg = {}
    if os.environ.get("KDEBUG", "0") == "1":
        for nm, shape in (("d_xall", (128, 16384)), ("d_qkT", (128, 4096)),
                          ("d_vaug", (128, 16 * 136)), ("d_es", (128, 16)),
                          ("d_aosc", (128, 2048)), ("d_aofull", (128, 8 * TOK)),
                          ("d_x1f", (128, 8 * TOK)), ("d_hb", (128, 8192)),
                          ("d_actt", (128, 8192)), ("d_rows", (1, 24 * TOK)),
                          ("d_denA", (128, 512)), ("d_denB", (128, 512)),
                          ("d_aos00", (128, 512)), ("d_ex00", (128, 1024)),
                          ("d_dn00", (128, 512))):
            dbg[nm] = nc.dram_tensor(nm, list(shape), F32, kind="ExternalOutput")
    ag_in = nc.dram_tensor("ag_in", [1024, TOK], BF, kind="Internal")
    ag_out = nc.dram_tensor("ag_out", [8192, TOK], BF, kind="Internal",
                            addr_space="Shared")
    a2a_in = nc.dram_tensor("a2a_in", [1024, TOK], BF, kind="Internal")
    a2a_out = nc.dram_tensor("a2a_out", [1024, TOK], BF, kind="Internal")

    with tile.TileContext(nc) as tc:
        _emit(nc, tc, tin, t_out, ag_in, ag_out, a2a_in, a2a_out, sc, dbg)
    nc.compile()
    return nc


def _ln_rows(nc, mu, st, tmp, sx, sx2, n, epsap):
    """mu = sx/n; st = 1/sqrt(var+eps) with var = sx2/n - mu^2."""
    v, s = nc.vector, nc.scalar
    v.tensor_scalar(mu, sx, 1.0 / n, None, Alu.mult)
    v.tensor_tensor(st, mu, mu, Alu.mult)
    v.tensor_scalar(tmp, sx2, 1.0 / n, None, Alu.mult)
    v.tensor_tensor(st, tmp, st, Alu.subtract)
    s.activation(st, st, Act.Ln, bias=epsap)
    s.activation(st, st, Act.Exp, scale=-0.5)


def _emit(nc, tc, tin, t_out, ag_in, ag_out, a2a_in, a2a_out, sc, dbg):
    v = nc.vector
    s = nc.scalar
    g = nc.gpsimd
    te = nc.tensor
    dma = nc.sync.dma_start
    c0, c1, c2, c3a, c4a, c5, c6, c7 = sc["coeffs"]
    bknot = sc["bknot"]
    RG = [list(range(NCORES))]

    with tc.tile_pool(name="persist", bufs=1) as P, \
         tc.tile_pool(name="consts", bufs=1) as C, \
         tc.tile_pool(name="rows", bufs=1) as R:

        # persistent tiles
        onesr = P.tile([1, 64], BF, tag="onesr")
        xt = P.tile([128, 8 * TOK], F32, tag="xt")
        qkT = P.tile([128, 4096], BF, tag="qkT")
        vaug = P.tile([128, 16 * 136], BF, tag="vaug")
        es = P.tile([128, 16], F32, tag="es")
        aosc = P.tile([128, 2048], BF, tag="aosc")
        aofull = P.tile([128, 8 * TOK], BF, tag="aofull")
        x1f = P.tile([128, 8 * TOK], F32, tag="x1f")
        x1b = P.tile([128, 8 * TOK], BF, tag="x1b")
        actt = P.tile([128, 8192], BF, tag="actt")
        outt = P.tile([128, 8 * TOK], F32, tag="outt")

        # constants
        sm = {}
        for nm, t in tin.items():
            if nm in ("xT", "wff1", "wff2", "wep1", "wout"):
                continue
            sm[nm] = C.tile(list(t.shape), t.dtype, tag=nm, name="sm_" + nm)
            dma(out=sm[nm][:], in_=t.ap())
        ones32, onesb, tri = sm["ones32"], sm["onesb"], sm["tri"]
        bvb = C.tile([128, 136], F32, tag="bvb")
        g.partition_broadcast(bvb[:], sm["b_v"][0:1, :])
        cst = C.tile([128, 5], F32, tag="cst")
        v.memset(cst[:, 0:1], EPS)
        v.memset(cst[:, 1:2], -sc["ent_b"])
        v.memset(cst[:, 2:3], -sc["ep2_b"])
        v.memset(cst[:, 3:4], -sc["bknot"])
        v.memset(cst[:, 4:5], sc["bknot"])

        v.memset(onesr[:], 1.0)
        rows = R.tile([1, 24 * TOK], F32, tag="rows")
        rs = lambda k: rows[0:1, k * TOK:(k + 1) * TOK]
        denpA = R.tile([128, 512], F32, tag="denpA")
        denpB = R.tile([128, 512], F32, tag="denpB")

        dma(out=xt[:], in_=tin["xT"].ap())

        # ============ Phase 1: LN(x) -> xl; AllGather ============
        XA_cm = tc.tile_pool(name="xa_pool", bufs=1)
        XA = XA_cm.__enter__()
        xall = XA.tile([128, 16384], BF, tag="xall")
        with tc.tile_pool(name="ps_r1", bufs=1, space="PSUM") as PSR, \
             tc.tile_pool(name="tmp1", bufs=2) as TMP:
            sx = PSR.tile([1, TOK], F32, tag="sx1p")
            sx2 = PSR.tile([1, TOK], F32, tag="sx2p")
            sx, sx2 = sx[:], sx2[:]
            for kc in range(8):
                te.matmul(sx, ones32[:], xt[:, TOK * kc:TOK * (kc + 1)],
                          start=(kc == 0), stop=(kc == 7))
            xsq = TMP.tile([128, TOK], F32, tag="xsq")
            for kc in range(8):
                v.tensor_tensor(xsq[:], xt[:, TOK * kc:TOK * (kc + 1)],
                                xt[:, TOK * kc:TOK * (kc + 1)], Alu.mult)
                te.matmul(sx2, ones32[:], xsq[:],
                          start=(kc == 0), stop=(kc == 7))
            _ln_rows(nc, rs(0), rs(1), rs(2), sx, sx2, D, cst[0:1, 0:1])
            mu_b = TMP.tile([128, TOK], F32, tag="mu_b")
            s_b = TMP.tile([128, TOK], F32, tag="s_b")
            g.partition_broadcast(mu_b[:], rs(0))
            g.partition_broadcast(s_b[:], rs(1))
            tm = TMP.tile([128, TOK], F32, tag="tm")
            for kc in range(8):
                xlc = TMP.tile([128, TOK], BF, tag="xlc")
                v.tensor_tensor(tm[:], xt[:, TOK * kc:TOK * (kc + 1)],
                                mu_b[:], Alu.subtract)
                v.tensor_tensor(tm[:], tm[:], s_b[:], Alu.mult)
                v.tensor_scalar(xlc[:], tm[:],
                                sm["lnw"][:, kc:kc + 1], sm["lnb"][:, kc:kc + 1],
                                Alu.mult, Alu.add)
                dma(out=ag_in.ap()[128 * kc:128 * (kc + 1), :], in_=xlc[:])
        g.collective_compute("AllGather", Alu.bypass, replica_groups=RG,
                             ins=[ag_in.ap()], outs=[ag_out.ap()])
        for kc in range(8):
            for r in range(NCORES):
                dma(out=xall[:, 2048 * kc + TOK * r:2048 * kc + TOK * (r + 1)],
                    in_=ag_out.ap()[1024 * r + 128 * kc:
                                    1024 * r + 128 * (kc + 1), :])

        # ============ Phase 2: QKV + ent + V ============
        with tc.tile_pool(name="wq_pool", bufs=1) as WQ, \
             tc.tile_pool(name="ps_qk", bufs=2, space="PSUM") as PSQ, \
             tc.tile_pool(name="ps_ev", bufs=3, space="PSUM") as PSV:
            went_s = WQ.tile([128, 8], BF, tag="went_s")
            dma(out=went_s[:], in_=tin["went"].ap())
            pse = PSV.tile([128, 16], F32, tag="pse", bufs=1)
            for tch in range(16):
                for kc in range(8):
                    te.matmul(
                        pse[:, tch:tch + 1],
                        xall[:, 2048 * kc + 128 * tch:2048 * kc + 128 * (tch + 1)],
                        went_s[:, kc:kc + 1],
                        start=(kc == 0), stop=(kc == 7))
            est = WQ.tile([128, 16], F32, tag="est")
            s.activation(est[:], pse[:], Act.Exp, bias=cst[:, 1:2], scale=-1.0)
            v.tensor_scalar(est[:], est[:], 1.0, None, Alu.add)
            v.reciprocal(es[:], est[:])
            v.tensor_scalar(es[:], es[:], 0.1, 2.0, Alu.max, Alu.min)
            wv_s = WQ.tile([128, 8 * 136], BF, tag="wv_s")
            dma(out=wv_s[:], in_=tin["wv"].ap())
            for tch in range(16):
                psv = PSV.tile([128, 136], F32, tag="psv", bufs=2)
                for kc in range(8):
                    te.matmul(
                        psv[:],
                        xall[:, 2048 * kc + 128 * tch:2048 * kc + 128 * (tch + 1)],
                        wv_s[:, 136 * kc:136 * (kc + 1)],
                        start=(kc == 0), stop=(kc == 7))
                vt = vaug[:, 136 * tch:136 * (tch + 1)]
                v.tensor_tensor(vt, psv[:], bvb[:], Alu.add)
                v.tensor_scalar(vt, vt, es[:, tch:tch + 1], None, Alu.mult)
                for lh in range(HPC):
                    v.memset(vaug[:, 136 * tch + 68 * lh + 64:
                                  136 * tch + 68 * lh + 65], 1.0)

            wqk_s = WQ.tile([128, 2048], BF, tag="wqk_s")
            dma(out=wqk_s[:], in_=tin["wqk"].ap())
            for of in range(2):
                for w in range(4):
                    ps = PSQ.tile([128, 512], F32, tag="psqk")
                    for kc in range(8):
                        te.matmul(
                            ps[:],
                            wqk_s[:, (of * 8 + kc) * 128:(of * 8 + kc + 1) * 128],
                            xall[:, 2048 * kc + 512 * w:2048 * kc + 512 * (w + 1)],
                            start=(kc == 0), stop=(kc == 7))
                    v.tensor_scalar(
                        qkT[:, 2048 * of + 512 * w:2048 * of + 512 * (w + 1)],
                        ps[:], sm["b_qk"][:, of:of + 1], None, Alu.add)

        if dbg:
            with tc.tile_pool(name="dbgx", bufs=2) as DBGX:
                for qq in range(8):
                    cvx = DBGX.tile([128, 2048], F32, tag="cvx")
                    v.tensor_copy(cvx[:], xall[:, 2048 * qq:2048 * (qq + 1)])
                    dma(out=dbg["d_xall"].ap()[:, 2048 * qq:2048 * (qq + 1)],
                        in_=cvx[:])
        XA_cm.__exit__(None, None, None)

        # ============ Phase 3: attention ============
        att_stash = []
        with tc.tile_pool(name="ps_sc", bufs=2, space="PSUM") as PSS, \
             tc.tile_pool(name="ps_ao", bufs=2, space="PSUM") as PSA, \
             tc.tile_pool(name="att_sb", bufs=3) as ASB, \
             tc.tile_pool(name="ao_sb", bufs=8) as AOSB:
            for lh in range(HPC):
                hq = qkT[64 * lh:64 * (lh + 1), 0:2048]
                hk = qkT[64 * lh:64 * (lh + 1), 2048:4096]
                for G in range(4):
                    nkb = 4 * G + 4
                    ao = PSA.tile([65, 512], F32, tag="ao")
                    for pj in range(nkb // 2):
                        ps = PSS.tile([128, 1024], F32, tag="ps_sc")
                        ex = ASB.tile([128, 1024], BF, tag="ex")
                        for half in range(2):
                            kb = 2 * pj + half
                            te.matmul(ps[:, 512 * half:512 * (half + 1)],
                                      hk[:, 128 * kb:128 * (kb + 1)],
                                      hq[:, 512 * G:512 * (G + 1)],
                                      start=True, stop=True)
                        s.activation(ex[:], ps[:], Act.Exp)
                        if dbg and lh == 0 and G == 0 and pj == 0:
                            exd = ASB.tile([128, 1024], F32, tag="exd")
                            v.tensor_copy(exd[:], ex[:])
                            dma(out=dbg["d_ex00"].ap(), in_=exd[:])
                        for half in range(2):
                            kb = 2 * pj + half
                            j = kb - 4 * G
                            exh = ex[:, 512 * half:512 * (half + 1)]
                            if 0 <= j < 4:
                                v.tensor_tensor(
                                    exh, exh, tri[:, 512 * j:512 * (j + 1)],
                                    Alu.mult)
                            te.matmul(
                                ao[:],
                                vaug[:, 136 * kb + 68 * lh:
                                     136 * kb + 68 * lh + 65],
                                exh,
                                start=(kb == 0), stop=(kb == nkb - 1))
                    aos = AOSB.tile([65, 512], F32, tag="aos")
                    s.copy(aos[:], ao[0:65, :])
                    dent = denpA if lh == 0 else denpB
                    v.tensor_copy(dent[32 * G:32 * G + 1, :], aos[64:65, :])
                    att_stash.append((lh, G, aos))
            v.reciprocal(denpA[:], denpA[:])
            v.reciprocal(denpB[:], denpB[:])
            for lh, G, aos in att_stash:
                rrow = ASB.tile([1, 512], BF, tag="rrow")
                dent = denpA if lh == 0 else denpB
                v.tensor_copy(rrow[0:1, :], dent[32 * G:32 * G + 1, :])
                rbp = PSA.tile([64, 512], F32, tag="rbp")
                te.matmul(rbp[:], onesr[:], rrow[:], start=True, stop=True)
                v.tensor_tensor(
                    aosc[64 * lh:64 * (lh + 1), 512 * G:512 * (G + 1)],
                    aos[0:64, :], rbp[:], Alu.mult)

        # ============ Phase 4: AllToAll ============
        for r in range(NCORES):
            dma(out=a2a_in.ap()[128 * r:128 * (r + 1), :],
                in_=aosc[:, TOK * r:TOK * (r + 1)])
        g.collective_compute("AllToAll", Alu.bypass, replica_groups=RG,
                             ins=[a2a_in.ap()], outs=[a2a_out.ap()])
        for r in range(NCORES):
            dma(out=aofull[:, TOK * r:TOK * (r + 1)],
                in_=a2a_out.ap()[128 * r:128 * (r + 1), :])

        # ============ Phase 5: out proj + norm1 ============
        with tc.tile_pool(name="wo_pool", bufs=1) as WO, \
             tc.tile_pool(name="ps_out", bufs=3, space="PSUM") as PSO, \
             tc.tile_pool(name="ps_r2", bufs=1, space="PSUM") as PSR2, \
             tc.tile_pool(name="tmp2", bufs=2) as TMP2:
            wout_s = WO.tile([128, 8192], BF, tag="wout_s")
            for qq in range(4):
                [nc.sync, nc.gpsimd, nc.sync, nc.gpsimd][qq].dma_start(
                    out=wout_s[:, 2048 * qq:2048 * (qq + 1)],
                    in_=tin["wout"].ap()[:, 2048 * qq:2048 * (qq + 1)])
            for of in range(8):
                ps = PSO.tile([128, TOK], F32, tag="ps_out")
                for kc in range(8):
                    te.matmul(
                        ps[:],
                        wout_s[:, (of * 8 + kc) * 128:(of * 8 + kc + 1) * 128],
                        aofull[:, TOK * kc:TOK * (kc + 1)],
                        start=(kc == 0), stop=(kc == 7))
                v.scalar_tensor_tensor(xt[:, TOK * of:TOK * (of + 1)],
                                       ps[:], sm["b_out"][:, of:of + 1],
                                       xt[:, TOK * of:TOK * (of + 1)],
                                       Alu.add, Alu.add)
            _ln_full(nc, tc, TMP2, PSR2, rows, xt, x1f, x1b, ones32,
                     sm["n1w"], sm["n1b"], cst[0:1, 0:1])

        # ============ Phase 6: ff1 + spline + ep path ============
        with tc.tile_pool(name="w1_pool", bufs=3) as W1, \
             tc.tile_pool(name="ps_h", bufs=2, space="PSUM") as PSH, \
             tc.tile_pool(name="ps_r3", bufs=1, space="PSUM") as PSR3, \
             tc.tile_pool(name="tmp3", bufs=1) as TMP3:
            hb = TMP3.tile([128, 8192], BF, tag="hb")
            t_sh = PSR3.tile([1, TOK], F32, tag="shp")
            t_sh2 = PSR3.tile([1, TOK], F32, tag="sh2p")
            t_se1 = PSR3.tile([1, TOK], F32, tag="se1p")
            t_se2 = PSR3.tile([1, TOK], F32, tag="se2p")
            t_pse2 = PSR3.tile([1, TOK], F32, tag="pse2p")
            sh, sh2, se1, se2, pse2 = (t_sh[:], t_sh2[:], t_se1[:],
                                       t_se2[:], t_pse2[:])
            hsqp = TMP3.tile([128, TOK], BF, tag="hsqp")
            for c in range(32):
                w1t = W1.tile([128, 1024], BF, tag="w1t", bufs=6)
                dmae = [nc.sync, nc.gpsimd][c % 2].dma_start
                dmae(out=w1t[:], in_=tin["wff1"].ap()[:, 1024 * c:1024 * (c + 1)])
                ps = PSH.tile([128, TOK], F32, tag="ps_h")
                for kc in range(8):
                    te.matmul(ps[:], w1t[:, 128 * kc:128 * (kc + 1)],
                              x1b[:, TOK * kc:TOK * (kc + 1)],
                              start=(kc == 0), stop=(kc == 7))
                hs = hb[:, TOK * c:TOK * (c + 1)]
                s.activation(hs, ps[:], Act.Identity,
                             bias=sm["b_ff1"][:, c:c + 1])
                g.tensor_tensor(hsqp[:], hs, hs, Alu.mult)
                te.matmul(sh, onesb[:], hs, start=(c == 0), stop=(c == 31))
                te.matmul(sh2, onesb[:], hsqp[:], start=(c == 0), stop=(c == 31))
            # ep path
            wep1_s = W1.tile([128, 8192], BF, tag="wep1_s", bufs=1)
            for qq in range(4):
                [nc.sync, nc.gpsimd, nc.sync, nc.gpsimd][qq].dma_start(
                    out=wep1_s[:, 2048 * qq:2048 * (qq + 1)],
                    in_=tin["wep1"].ap()[:, 2048 * qq:2048 * (qq + 1)])
            epb = TMP3.tile([128, 2 * TOK], BF, tag="epb")
            epsq = TMP3.tile([128, TOK], BF, tag="epsq")
            for of in range(2):
                ps = PSH.tile([128, TOK], F32, tag="ps_h")
                for kc in range(32):
                    te.matmul(
                        ps[:],
                        wep1_s[:, (of * 32 + kc) * 128:(of * 32 + kc + 1) * 128],
                        hb[:, TOK * kc:TOK * (kc + 1)],
                        start=(kc == 0), stop=(kc == 31))
                s.activation(epb[:, TOK * of:TOK * (of + 1)], ps[:],
                             Act.Identity, bias=sm["b_ep1"][:, of:of + 1])
                v.tensor_tensor(epsq[:], epb[:, TOK * of:TOK * (of + 1)],
                                epb[:, TOK * of:TOK * (of + 1)], Alu.mult)
                te.matmul(se1, onesb[:], epb[:, TOK * of:TOK * (of + 1)],
                          start=(of == 0), stop=(of == 1))
                te.matmul(se2, onesb[:], epsq[:],
                          start=(of == 0), stop=(of == 1))
            _ln_rows(nc, rs(3), rs(4), rs(5), se1, se2, D16, cst[0:1, 0:1])
            mue_b = TMP3.tile([128, TOK], F32, tag="mue_b")
            see_b = TMP3.tile([128, TOK], F32, tag="see_b")
            g.partition_broadcast(mue_b[:], rs(3))
            g.partition_broadcast(see_b[:], rs(4))
            relub = TMP3.tile([128, 2 * TOK], BF, tag="relub")
            tm3 = TMP3.tile([128, TOK], F32, tag="tm3")
            for of in range(2):
                v.tensor_tensor(tm3[:], epb[:, TOK * of:TOK * (of + 1)],
                                mue_b[:], Alu.subtract)
                v.tensor_tensor(tm3[:], tm3[:], see_b[:], Alu.mult)
                v.tensor_scalar(tm3[:], tm3[:], sm["eplw"][:, of:of + 1],
                                sm["eplb"][:, of:of + 1], Alu.mult, Alu.add)
                v.tensor_scalar(relub[:, TOK * of:TOK * (of + 1)], tm3[:],
                                0.0, None, Alu.max)
            for of in range(2):
                te.matmul(pse2, sm["wep2"][:, of:of + 1],
                          relub[:, TOK * of:TOK * (of + 1)],
                          start=(of == 0), stop=(of == 1))
            erow = rs(6)
            s.activation(erow, pse2, Act.Exp, bias=cst[0:1, 2:3], scale=-1.0)
            v.tensor_scalar(erow, erow, 1.0, None, Alu.add)
            v.reciprocal(erow, erow)
            v.tensor_scalar(erow, erow, 0.1, 1.0, Alu.mult, Alu.add)

            # spline per-token rows: mu_h (7), S (8)
            _spline_rows(nc, rs, sh, sh2, cst[0:1, 0:1])

            muh_b = TMP3.tile([128, TOK], F32, tag="muh_b")
            Sh_b = TMP3.tile([128, TOK], F32, tag="Sh_b")
            em_b = TMP3.tile([128, TOK], F32, tag="em_b")
            g.partition_broadcast(muh_b[:], rs(7))
            g.partition_broadcast(Sh_b[:], rs(8))
            g.partition_broadcast(em_b[:], rs(6))
            murep = TMP3.tile([128, 2048], BF, tag="murep")
            Srep = TMP3.tile([128, 2048], BF, tag="Srep")
            emrep = TMP3.tile([128, 2048], BF, tag="emrep")
            for (src, dst) in ((muh_b, murep), (Sh_b, Srep), (em_b, emrep)):
                v.tensor_copy(dst[:], src[:].unsqueeze(1).to_broadcast((128, 8, TOK)))

            with tc.tile_pool(name="spl", bufs=1) as SPL:
                for gi in range(4):
                    hbs = hb[:, 2048 * gi:2048 * (gi + 1)]
                    u = SPL.tile([128, 2048], BF, tag="u")
                    acc = SPL.tile([128, 2048], BF, tag="acc")
                    t1 = SPL.tile([128, 2048], BF, tag="t1")
                    t2 = SPL.tile([128, 2048], BF, tag="t2")
                    t3 = SPL.tile([128, 2048], BF, tag="t3")
                    v.tensor_tensor(u[:], hbs, murep[:], Alu.subtract)
                    v.tensor_tensor(u[:], u[:], Srep[:], Alu.mult)
                    v.tensor_scalar(u[:], u[:], -UDOM, UDOM, Alu.max, Alu.min)
                    v.tensor_tensor(t1[:], u[:], u[:], Alu.mult)
                    s.activation(t3[:], u[:], Act.Abs)
                    v.tensor_scalar(acc[:], t1[:], c2, c0, Alu.mult, Alu.add)
                    v.scalar_tensor_tensor(acc[:], u[:], c1, acc[:],
                                           Alu.mult, Alu.add)
                    v.scalar_tensor_tensor(acc[:], t3[:], c3a, acc[:],
                                           Alu.mult, Alu.add)
                    s.activation(t2[:], u[:], Act.Abs, bias=cst[:, 3:4])
                    v.scalar_tensor_tensor(acc[:], t2[:], c6, acc[:],
                                           Alu.mult, Alu.add)
                    s.activation(t2[:], u[:], Act.Abs, bias=cst[:, 4:5])
                    v.scalar_tensor_tensor(acc[:], t2[:], c7, acc[:],
                                           Alu.mult, Alu.add)
                    v.tensor_tensor(acc[:], acc[:], emrep[:], Alu.mult)
                    v.tensor_scalar(actt[:, 2048 * gi:2048 * (gi + 1)],
                                    acc[:], 1.0, -1.0, Alu.min, Alu.max)
            if dbg:
                with tc.tile_pool(name="dbgh", bufs=2) as DBGH:
                    for qq in range(4):
                        cvh = DBGH.tile([128, 2048], F32, tag="cvh")
                        v.tensor_copy(cvh[:], hb[:, 2048 * qq:2048 * (qq + 1)])
                        dma(out=dbg["d_hb"].ap()[:, 2048 * qq:2048 * (qq + 1)],
                            in_=cvh[:])
                        cvh2 = DBGH.tile([128, 2048], F32, tag="cvh2")
                        v.tensor_copy(cvh2[:], actt[:, 2048 * qq:2048 * (qq + 1)])
                        dma(out=dbg["d_actt"].ap()[:, 2048 * qq:2048 * (qq + 1)],
                            in_=cvh2[:])

        # ============ Phase 7: ff2 + norm2 ============
        with tc.tile_pool(name="w2_pool", bufs=3) as W2, \
             tc.tile_pool(name="ps_f2", bufs=3, space="PSUM") as PSF, \
             tc.tile_pool(name="ps_r4", bufs=1, space="PSUM") as PSR4, \
             tc.tile_pool(name="tmp4", bufs=2) as TMP4:
            r2 = TMP4.tile([128, 8 * TOK], F32, tag="r2")
            for of in range(8):
                w2t = W2.tile([128, 4096], BF, tag="w2t", bufs=4)
                dmae = [nc.sync, nc.gpsimd][of % 2].dma_start
                dmae(out=w2t[:], in_=tin["wff2"].ap()[:, 4096 * of:4096 * (of + 1)])
                ps = PSF.tile([128, TOK], F32, tag="ps_f2")
                for kc in range(32):
                    te.matmul(ps[:], w2t[:, 128 * kc:128 * (kc + 1)],
                              actt[:, TOK * kc:TOK * (kc + 1)],
                              start=(kc == 0), stop=(kc == 31))
                v.scalar_tensor_tensor(r2[:, TOK * of:TOK * (of + 1)],
                                       ps[:], sm["b_ff2"][:, of:of + 1],
                                       x1f[:, TOK * of:TOK * (of + 1)],
                                       Alu.add, Alu.add)
            _ln_full(nc, tc, TMP4, PSR4, rows, r2, outt, None, ones32,
                     sm["n2w"], sm["n2b"], cst[0:1, 0:1])
        dma(out=t_out.ap(), in_=outt[:])
        if dbg:
            with tc.tile_pool(name="dbgp", bufs=2) as DBG:
                def dump(name, tile_ap, width):
                    nch = max(1, width // 2048)
                    w = width // nch
                    for qq in range(nch):
                        cv = DBG.tile([128, w], F32, tag="cv",
                                      name=f"cv{name}{qq}")
                        v.tensor_copy(cv[:], tile_ap[:, w * qq:w * (qq + 1)])
                        dma(out=dbg[name].ap()[:, w * qq:w * (qq + 1)],
                            in_=cv[:])
                dump("d_qkT", qkT[:], 4096)
                dump("d_vaug", vaug[:], 16 * 136)
                dump("d_aosc", aosc[:], 2048)
                dump("d_aofull", aofull[:], 8 * TOK)
                dma(out=dbg["d_es"].ap(), in_=es[:])
                dma(out=dbg["d_x1f"].ap(), in_=x1f[:])
                dma(out=dbg["d_rows"].ap(), in_=rows[:])
                dma(out=dbg["d_denA"].ap(), in_=denpA[:])
                dma(out=dbg["d_denB"].ap(), in_=denpB[:])


def _spline_rows(nc, rs, sh, sh2, epsap):
    """rs(7) = mu_h, rs(8) = S = 1/(sqrt(var+eps)*(norm+1)),
    norm = sqrt(FF*var/(var+eps) + eps)."""
    v, s = nc.vector, nc.scalar
    mu = rs(7)
    S_ = rs(8)
    var = rs(13)
    t1 = rs(14)
    t2 = rs(15)
    v.tensor_scalar(mu, sh, 1.0 / FF, None, Alu.mult)
    v.tensor_tensor(var, mu, mu, Alu.mult)
    v.tensor_scalar(t1, sh2, 1.0 / FF, None, Alu.mult)
    v.tensor_tensor(var, t1, var, Alu.subtract)
    s.activation(t1, var, Act.Ln, bias=epsap)
    s.activation(t1, t1, Act.Exp, scale=0.5)          # sqrt(var+eps)
    v.tensor_scalar(t2, var, EPS, None, Alu.add)
    v.reciprocal(t2, t2)
    v.tensor_tensor(t2, t2, var, Alu.mult)
    v.tensor_scalar(t2, t2, float(FF), None, Alu.mult)
    s.activation(t2, t2, Act.Ln, bias=epsap)
    s.activation(t2, t2, Act.Exp, scale=0.5)          # norm
    v.tensor_scalar(t2, t2, 1.0, None, Alu.add)
    v.tensor_tensor(t2, t2, t1, Alu.mult)
    v.reciprocal(S_, t2)


def _ln_full(nc, tc, TMP, PSR, rows, src, dstf, dstb, ones32, wcol, bcol, epsap):
    v, s, g, te = nc.vector, nc.scalar, nc.gpsimd, nc.tensor
    T = TOK
    rs = lambda k: rows[0:1, k * T:(k + 1) * T]
    t_sx = PSR.tile([1, T], F32, tag="lnsxp")
    t_sx2 = PSR.tile([1, T], F32, tag="lnsx2p")
    sx, sx2 = t_sx[:], t_sx2[:]
    for kc in range(8):
        te.matmul(sx, ones32[:], src[:, T * kc:T * (kc + 1)],
                  start=(kc == 0), stop=(kc == 7))
    xsq = TMP.tile([128, T], F32, tag="lnxsq")
    for kc in range(8):
        v.tensor_tensor(xsq[:], src[:, T * kc:T * (kc + 1)],
                        src[:, T * kc:T * (kc + 1)], Alu.mult)
        te.matmul(sx2, ones32[:], xsq[:], start=(kc == 0), stop=(kc == 7))
    _ln_rows(nc, rs(9), rs(10), rs(11), sx, sx2, D, epsap)
    mu_b = TMP.tile([128, T], F32, tag="lnmu_b")
    s_b = TMP.tile([128, T], F32, tag="lns_b")
    g.partition_broadcast(mu_b[:], rs(9))
    g.partition_broadcast(s_b[:], rs(10))
    tm = TMP.tile([128, T], F32, tag="lntm")
    for kc in range(8):
        v.tensor_tensor(tm[:], src[:, T * kc:T * (kc + 1)], mu_b[:],
                        Alu.subtract)
        v.tensor_tensor(tm[:], tm[:], s_b[:], Alu.mult)
        v.tensor_scalar(dstf[:, T * kc:T * (kc + 1)], tm[:],
                        wcol[:, kc:kc + 1], bcol[:, kc:kc + 1],
                        Alu.mult, Alu.add)
        if dstb is not None:
            v.tensor_copy(dstb[:, T * kc:T * (kc + 1)],
                          dstf[:, T * kc:T * (kc + 1)])


# ----------------------------------------------------------------------------
# Entry point
# ----------------------------------------------------------------------------

def kernel(**inputs):
    in_maps, sc = _prepare_inputs(inputs)
    key = hashlib.sha256(
        repr((sc["coeffs"], sc["bknot"], sc["ent_b"], sc["ep2_b"])).encode()
    ).hexdigest()
    if key not in _prog_cache:
        _prog_cache[key] = _build_program(sc)
    nc = _prog_cache[key]
    res = bass_utils.run_bass_kernel_spmd(nc, in_maps,
                                          core_ids=list(range(NCORES)))
    out = np.empty((1, S, D), np.float32)
    for c in range(NCORES):
        oc = np.asarray(res.results[c]["out"], np.float32)   # [128, 8*TOK]
        ot = oc.reshape(128, 8, TOK).transpose(1, 0, 2).reshape(D, TOK)
        out[0, c * TOK:(c + 1) * TOK, :] = ot.T
    return out



# revision 9
# speedup vs baseline: 1.8486x; 1.8486x over previous
"""Trainium2 8-core kernel for nn_EnhancedTransformerBlock.

SPMD: identical program on all 8 cores, only in_maps data differs.
  - Sequence-sharded everywhere except attention: core c owns tokens
    [256c, 256c+256), activations in T-layout [feature, token].
  - Attention head-sharded (2 of 16 heads per core, full sequence).
    AllGather of ln(x) (bf16) + the transposed entropy gate before QKV;
    AllToAll of per-head attention outputs back to sequence sharding.
  - All GEMMs bf16 (weights pre-transposed/packed host-side), fp32 PSUM.
  - Softmax: temperature (1/0.1) and 1/sqrt(hd) folded into Wq; unshifted
    exp (max |logit| ~35 fits fp32); denominator via appended ones-column
    on V; causal masking via duplicated triangle-mask multiply.
  - The two heads' K=64 score matmuls are emitted back-to-back at
    partition bases 0/64 so they occupy disjoint PE row-groups and run
    concurrently.
  - FFN shortcut: the reference's spline activation is
    clip(g(u)*(1+0.1*sigmoid(...)), -1, 1) with g(u) = softmax-weighted
    spl_w mixture.  |u| < 1 holds unconditionally (|lnh_i| <= ||lnh||),
    and when min_u g(u) >= 1-delta on that domain the clipped product is
    1 within delta for every element (the gate factor is >= 1).  Then
    act == 1, so ff == rowsum(ff2_w)+ff2_b is a constant vector folded
    into norm1's bias, and ff1/spline/ep-path/ff2 are all dead.  Checked
    host-side from the runtime knots/spl_w; falls back to the full
    kernel otherwise.
"""

import hashlib
import numpy as np

from concourse import bacc, tile, mybir
from concourse import bass_utils

dt = mybir.dt
BF = dt.bfloat16
F32 = dt.float32
NPBF = dt.np(BF)
Alu = mybir.AluOpType
Act = mybir.ActivationFunctionType

NCORES = 8
S = 2048
D = 1024
H = 16
HD = 64
FF = 4096
D16 = 256
TOK = S // NCORES            # 256 tokens per core
HPC = H // NCORES            # 2 heads per core
EPS = 1e-6
UDOM = 0.2                   # spline fit domain |u| <= UDOM (full path)
AGP = 1152                   # AllGather payload rows: 1024 xl + 128 esT

_prog_cache = {}


# ----------------------------------------------------------------------------
# Host-side: spline analysis
# ----------------------------------------------------------------------------

def _g_exact(u, knots, spl_w):
    d = np.abs(u[:, None] - knots[None, :])
    d = d / (d.max(-1, keepdims=True) + EPS)
    a = -5.0 * d
    a = a - a.max(-1, keepdims=True)
    e = np.exp(a)
    p = e / e.sum(-1, keepdims=True)
    return (p * spl_w).sum(-1)


def _const_act_ok(knots, spl_w):
    """True when clip(g(u)*(1+0.1 e), -1, 1) == 1 within 6e-3 for every
    reachable u (|u| < 1 holds unconditionally: |u|=|lnh_i|/(||lnh||+1))."""
    u = np.linspace(-1.0, 1.0, 50001)
    g = _g_exact(u, np.asarray(knots, np.float64), np.asarray(spl_w, np.float64))
    return float(g.min()) >= 1.0 - 6e-3 and float(g.max()) * 1.1 <= 1.25


def _fit_spline(knots, spl_w):
    """LSQ fit of g(u) on [-UDOM, UDOM] (full fallback path)."""
    k = np.asarray(knots, np.float64)
    w = np.asarray(spl_w, np.float64)
    pos = np.sort(np.abs(k[(np.abs(k) > 1e-9) & (np.abs(k) < UDOM * 0.9)]))
    b = float(pos[0]) if len(pos) else UDOM / 2.0
    u = np.linspace(-UDOM, UDOM, 20001)
    B = np.stack([np.ones_like(u), u, u * u, np.abs(u),
                  np.abs(u - b), np.abs(u + b)], -1)
    y = _g_exact(u, k, w)
    c, *_ = np.linalg.lstsq(B, y, rcond=None)
    err = float(np.abs(B @ c - y).max())
    cc = [float(c[0]), float(c[1]), float(c[2]), float(c[3]), 0.0, 0.0,
          float(c[4]), float(c[5])]
    return cc, b, err


# ----------------------------------------------------------------------------
# Host-side: weight packing
# ----------------------------------------------------------------------------

def _pack_lhsT(w_t, n_of, n_kc):
    """w_t: [K_total, M_total] ([in, out]) -> [128, n_of*n_kc*128], tile
    (of, kc) at cols [(of*n_kc+kc)*128 ...] = w_t[128kc:.., 128of:..]."""
    K_total, M_total = w_t.shape
    assert K_total == n_kc * 128 and M_total == n_of * 128
    out = np.empty((128, n_of * n_kc * 128), np.float32)
    for of in range(n_of):
        for kc in range(n_kc):
            out[:, (of * n_kc + kc) * 128:(of * n_kc + kc + 1) * 128] = \
                w_t[kc * 128:(kc + 1) * 128, of * 128:(of + 1) * 128]
    return np.ascontiguousarray(out)


def _col_pack(vec, n_chunks):
    return np.ascontiguousarray(
        np.asarray(vec, np.float32).reshape(n_chunks, 128).T)


def _make_tri2():
    """[128, 4*1024]: block j = [tri_j | tri_j], tri_j[kk, q] = (kk+128j <= q)."""
    out = np.zeros((128, 4 * 1024), np.float32)
    for j in range(4):
        kk = np.arange(128)[:, None] + 128 * j
        q = np.arange(512)[None, :]
        m = (kk <= q).astype(np.float32)
        out[:, 1024 * j:1024 * j + 512] = m
        out[:, 1024 * j + 512:1024 * (j + 1)] = m
    return out


def _prepare_fast(inputs):
    f = lambda k: np.asarray(inputs[k], np.float32)
    x = f("x").reshape(S, D)
    qkv_w, qkv_b = f("qkv_w"), f("qkv_b")
    out_w, out_b = f("out_w") * 0.1, f("out_b") * 0.1
    ent_w, ent_b = f("ent_w"), f("ent_b")
    ff2_w, ff2_b = f("ff2_w"), f("ff2_b")

    temp = (1.0 / np.sqrt(np.float32(HD))) / 0.1   # 1.25
    wq = qkv_w[0:D] * temp
    wk = qkv_w[D:2 * D]
    wv = qkv_w[2 * D:3 * D]
    bq = qkv_b[0:D] * temp
    bk = qkv_b[D:2 * D]
    bv = qkv_b[2 * D:3 * D]

    # act == 1 => ff = rowsum(ff2_w) + ff2_b, folded into norm1's bias.
    cvec = (ff2_w.astype(np.float64).sum(1) + ff2_b).astype(np.float32)

    shared = {
        "tri2": _make_tri2().astype(NPBF),
        "ones32": np.ones((128, 1), np.float32),
        "onesr": np.ones((1, 64), NPBF),
        "wout": _pack_lhsT(out_w.T, 8, 8).astype(NPBF),
        "went": np.ascontiguousarray(
            ent_w.reshape(8, 128).T).astype(NPBF),          # [128, 8]
        "b_out": _col_pack(out_b, 8),
        "lnw": _col_pack(f("ln_attn_w"), 8),
        "lnb": _col_pack(f("ln_attn_b"), 8),
        "n1w": _col_pack(f("norm1_w"), 8),
        "n1b": _col_pack(f("norm1_b") + cvec, 8),
        "n2w": _col_pack(f("norm2_w"), 8),
        "n2b": _col_pack(f("norm2_b"), 8),
    }

    scalars = {"ent_b": float(ent_b.reshape(-1)[0]), "fast": True}

    in_maps = []
    for c in range(NCORES):
        m = dict(shared)
        xc = x[c * TOK:(c + 1) * TOK]                        # [256, D]
        xT = np.ascontiguousarray(xc.T)                      # [D, 256]
        m["xT"] = np.ascontiguousarray(
            xT.reshape(8, 128, TOK).transpose(1, 0, 2).reshape(128, 8 * TOK))
        h0 = c * HPC
        wq_c = wq[h0 * HD:(h0 + HPC) * HD]                   # [128, D]
        wk_c = wk[h0 * HD:(h0 + HPC) * HD]
        wqk_t = np.concatenate([wq_c, wk_c], 0).T            # [D, 256]
        m["wqk"] = _pack_lhsT(wqk_t, 2, 8).astype(NPBF)
        m["b_qk"] = np.ascontiguousarray(np.stack(
            [bq[h0 * HD:(h0 + HPC) * HD],
             bk[h0 * HD:(h0 + HPC) * HD]], -1).astype(np.float32))
        wv_c = wv[h0 * HD:(h0 + HPC) * HD].T                 # [D, 128]
        wva = np.zeros((D, 136), np.float32)
        bva = np.zeros((1, 136), np.float32)
        for lh in range(HPC):
            wva[:, 68 * lh:68 * lh + 64] = wv_c[:, 64 * lh:64 * lh + 64]
            bva[0, 68 * lh:68 * lh + 64] = \
                bv[(h0 + lh) * HD:(h0 + lh + 1) * HD]
        m["wv"] = np.ascontiguousarray(
            wva.reshape(8, 128, 136).transpose(1, 0, 2).reshape(128, 8 * 136)
        ).astype(NPBF)
        m["b_v"] = bva
        in_maps.append(m)

    return in_maps, scalars


# ----------------------------------------------------------------------------
# Device program (fast path)
# ----------------------------------------------------------------------------

def _build_fast(sc):
    nc = bacc.Bacc("TRN2", target_bir_lowering=False, debug=False,
                   num_devices=NCORES)

    def din(name, shape, dtype):
        return nc.dram_tensor(name, list(shape), dtype, kind="ExternalInput")

    tin = {
        "xT": din("xT", (128, 8 * TOK), F32),
        "wqk": din("wqk", (128, 2048), BF),
        "wv": din("wv", (128, 8 * 136), BF),
        "went": din("went", (128, 8), BF),
        "wout": din("wout", (128, 8192), BF),
        "tri2": din("tri2", (128, 4096), BF),
        "ones32": din("ones32", (128, 1), F32),
        "onesr": din("onesr", (1, 64), BF),
        "b_qk": din("b_qk", (128, 2), F32),
        "b_v": din("b_v", (1, 136), F32),
        "b_out": din("b_out", (128, 8), F32),
        "lnw": din("lnw", (128, 8), F32),
        "lnb": din("lnb", (128, 8), F32),
        "n1w": din("n1w", (128, 8), F32),
        "n1b": din("n1b", (128, 8), F32),
        "n2w": din("n2w", (128, 8), F32),
        "n2b": din("n2b", (128, 8), F32),
    }
    t_out = nc.dram_tensor("out", [128, 8 * TOK], F32, kind="ExternalOutput")
    ag_in = nc.dram_tensor("ag_in", [AGP, TOK], BF, kind="Internal")
    ag_out = nc.dram_tensor("ag_out", [AGP * NCORES, TOK], BF, kind="Internal",
                            addr_space="Shared")
    a2a_in = nc.dram_tensor("a2a_in", [1024, TOK], BF, kind="Internal")
    a2a_out = nc.dram_tensor("a2a_out", [1024, TOK], BF, kind="Internal")

    with tile.TileContext(nc) as tc:
        _emit_fast(nc, tc, tin, t_out, ag_in, ag_out, a2a_in, a2a_out, sc)
    nc.compile()
    return nc


def _ln_rows(nc, mu, st, tmp, sx, sx2, n, epsap):
    """mu = sx/n; st = 1/sqrt(var+eps) with var = sx2/n - mu^2."""
    v, s = nc.vector, nc.scalar
    v.tensor_scalar(mu, sx, 1.0 / n, None, Alu.mult)
    v.tensor_tensor(st, mu, mu, Alu.mult)
    v.tensor_scalar(tmp, sx2, 1.0 / n, None, Alu.mult)
    v.tensor_tensor(st, tmp, st, Alu.subtract)
    s.activation(st, st, Act.Ln, bias=epsap)
    s.activation(st, st, Act.Exp, scale=-0.5)


def _ln_full(nc, tc, TMP, PSR, rows, src, dstf, ones32, wcol, bcol, epsap,
             rbase, out_dma=None):
    """dstf = LN(src)*wcol + bcol, chunked [128, TOK] x 8.
    If out_dma=(dram, queues), each chunk is DMA'd out as it is produced."""
    v, s, g, te = nc.vector, nc.scalar, nc.gpsimd, nc.tensor
    T = TOK
    rs = lambda k: rows[0:1, (rbase + k) * T:(rbase + k + 1) * T]
    t_sx = PSR.tile([1, T], F32, tag=f"lnsxp{rbase}")
    t_sx2 = PSR.tile([1, T], F32, tag=f"lnsx2p{rbase}")
    sx, sx2 = t_sx[:], t_sx2[:]
    for kc in range(8):
        te.matmul(sx, ones32[:], src[:, T * kc:T * (kc + 1)],
                  start=(kc == 0), stop=(kc == 7))
    xsq = TMP.tile([128, T], F32, tag=f"lnxsq{rbase}")
    for kc in range(8):
        v.tensor_tensor(xsq[:], src[:, T * kc:T * (kc + 1)],
                        src[:, T * kc:T * (kc + 1)], Alu.mult)
        te.matmul(sx2, ones32[:], xsq[:], start=(kc == 0), stop=(kc == 7))
    _ln_rows(nc, rs(0), rs(1), rs(2), sx, sx2, D, epsap)
    mu_b = TMP.tile([128, T], F32, tag=f"lnmu_b{rbase}")
    s_b = TMP.tile([128, T], F32, tag=f"lns_b{rbase}")
    g.partition_broadcast(mu_b[:], rs(0))
    g.partition_broadcast(s_b[:], rs(1))
    tm = TMP.tile([128, T], F32, tag=f"lntm{rbase}")
    for kc in range(8):
        v.tensor_tensor(tm[:], src[:, T * kc:T * (kc + 1)], mu_b[:],
                        Alu.subtract)
        v.tensor_tensor(tm[:], tm[:], s_b[:], Alu.mult)
        v.tensor_scalar(dstf[:, T * kc:T * (kc + 1)], tm[:],
                        wcol[:, kc:kc + 1], bcol[:, kc:kc + 1],
                        Alu.mult, Alu.add)
        if out_dma is not None:
            dram, queues = out_dma
            queues[kc % len(queues)].dma_start(
                out=dram.ap()[:, T * kc:T * (kc + 1)],
                in_=dstf[:, T * kc:T * (kc + 1)])


def _emit_fast(nc, tc, tin, t_out, ag_in, ag_out, a2a_in, a2a_out, sc):
    v = nc.vector
    s = nc.scalar
    g = nc.gpsimd
    te = nc.tensor
    dma = nc.sync.dma_start
    RG = [list(range(NCORES))]
    QS = [nc.sync, nc.gpsimd, nc.scalar]

    with tc.tile_pool(name="persist", bufs=1) as P, \
         tc.tile_pool(name="consts", bufs=1) as C, \
         tc.tile_pool(name="rows", bufs=1) as R:

        # ---- persistent tiles
        xt = P.tile([128, 8 * TOK], F32, tag="xt")
        xlb = P.tile([128, 8 * TOK], BF, tag="xlb")
        xall = P.tile([128, 16384], BF, tag="xall")
        qkT = P.tile([128, 4096], BF, tag="qkT")
        vaug = P.tile([128, 16 * 136], BF, tag="vaug")
        es32 = P.tile([128, 16], F32, tag="es32")
        aosc = P.tile([128, 2048], BF, tag="aosc")
        aofull = P.tile([128, 8 * TOK], BF, tag="aofull")
        x1f = P.tile([128, 8 * TOK], F32, tag="x1f")
        outt = P.tile([128, 8 * TOK], F32, tag="outt")
        wqk_s = P.tile([128, 2048], BF, tag="wqk_s")
        wv_s = P.tile([128, 8 * 136], BF, tag="wv_s")
        wout_s = P.tile([128, 8192], BF, tag="wout_s")
        tri2_s = P.tile([128, 4096], BF, tag="tri2_s")
        denpA = P.tile([128, 512], F32, tag="denpA")
        denpB = P.tile([128, 512], F32, tag="denpB")

        # ---- constants (small)
        sm = {}
        for nm in ("went", "ones32", "onesr", "b_qk", "b_v", "b_out",
                   "lnw", "lnb", "n1w", "n1b", "n2w", "n2b"):
            t = tin[nm]
            sm[nm] = C.tile(list(t.shape), t.dtype, tag=nm, name="sm_" + nm)
            dma(out=sm[nm][:], in_=t.ap())
        ones32, onesr = sm["ones32"], sm["onesr"]
        bvb = C.tile([128, 136], F32, tag="bvb")
        g.partition_broadcast(bvb[:], sm["b_v"][0:1, :])
        cst = C.tile([128, 2], F32, tag="cst")
        v.memset(cst[:, 0:1], EPS)
        v.memset(cst[:, 1:2], -sc["ent_b"])
        epsap = cst[0:1, 0:1]

        rows = R.tile([1, 16 * TOK], F32, tag="rows")
        rs = lambda k: rows[0:1, k * TOK:(k + 1) * TOK]

        # ---- input DMAs (spread queues; big weights prefetch early)
        nc.sync.dma_start(out=xt[:, 0:1024], in_=tin["xT"].ap()[:, 0:1024])
        nc.gpsimd.dma_start(out=xt[:, 1024:2048],
                            in_=tin["xT"].ap()[:, 1024:2048])
        nc.scalar.dma_start(out=wqk_s[:], in_=tin["wqk"].ap())
        nc.scalar.dma_start(out=wv_s[:], in_=tin["wv"].ap())
        for qq in range(4):
            QS[qq % 3].dma_start(out=wout_s[:, 2048 * qq:2048 * (qq + 1)],
                                 in_=tin["wout"].ap()[:, 2048 * qq:2048 * (qq + 1)])
        nc.sync.dma_start(out=tri2_s[:, 0:2048],
                          in_=tin["tri2"].ap()[:, 0:2048])
        nc.gpsimd.dma_start(out=tri2_s[:, 2048:4096],
                            in_=tin["tri2"].ap()[:, 2048:4096])

        # ============ Phase 1: LN(x) -> xl; ent -> esT; AllGather ============
        with tc.tile_pool(name="ps_r1", bufs=1, space="PSUM") as PSR, \
             tc.tile_pool(name="tmp1", bufs=2) as TMP:
            t_sx = PSR.tile([1, TOK], F32, tag="sx1p")
            t_sx2 = PSR.tile([1, TOK], F32, tag="sx2p")
            sx, sx2 = t_sx[:], t_sx2[:]
            for kc in range(8):
                te.matmul(sx, ones32[:], xt[:, TOK * kc:TOK * (kc + 1)],
                          start=(kc == 0), stop=(kc == 7))
            xsq = TMP.tile([128, TOK], F32, tag="xsq")
            for kc in range(8):
                v.tensor_tensor(xsq[:], xt[:, TOK * kc:TOK * (kc + 1)],
                                xt[:, TOK * kc:TOK * (kc + 1)], Alu.mult)
                te.matmul(sx2, ones32[:], xsq[:],
                          start=(kc == 0), stop=(kc == 7))
            _ln_rows(nc, rs(0), rs(1), rs(2), sx, sx2, D, epsap)
            mu_b = TMP.tile([128, TOK], F32, tag="mu_b")
            s_b = TMP.tile([128, TOK], F32, tag="s_b")
            g.partition_broadcast(mu_b[:], rs(0))
            g.partition_broadcast(s_b[:], rs(1))
            tm = TMP.tile([128, TOK], F32, tag="tm")
            for kc in range(8):
                v.tensor_tensor(tm[:], xt[:, TOK * kc:TOK * (kc + 1)],
                                mu_b[:], Alu.subtract)
                v.tensor_tensor(tm[:], tm[:], s_b[:], Alu.mult)
                v.tensor_scalar(xlb[:, TOK * kc:TOK * (kc + 1)], tm[:],
                                sm["lnw"][:, kc:kc + 1], sm["lnb"][:, kc:kc + 1],
                                Alu.mult, Alu.add)
                QS[kc % 3].dma_start(
                    out=ag_in.ap()[128 * kc:128 * (kc + 1), :],
                    in_=xlb[:, TOK * kc:TOK * (kc + 1)])
            # entropy gate for own tokens, token-partition orientation
            t_pse = PSR.tile([128, 2], F32, tag="psev")
            pse = t_pse[:]
            for t in range(2):
                for kc in range(8):
                    te.matmul(pse[:, t:t + 1],
                              xlb[:, TOK * kc + 128 * t:TOK * kc + 128 * (t + 1)],
                              sm["went"][:, kc:kc + 1],
                              start=(kc == 0), stop=(kc == 7))
            est = TMP.tile([128, 2], F32, tag="est")
            s.activation(est[:], pse, Act.Exp, bias=cst[:, 1:2], scale=-1.0)
            v.tensor_scalar(est[:], est[:], 1.0, None, Alu.add)
            v.reciprocal(est[:], est[:])
            esb = TMP.tile([128, 2], BF, tag="esb")
            v.tensor_scalar(esb[:], est[:], 0.1, None, Alu.max)
            dma(out=ag_in.ap()[1024:1152, 0:2], in_=esb[:])

        g.collective_compute("AllGather", Alu.bypass, replica_groups=RG,
                             ins=[ag_in.ap()], outs=[ag_out.ap()])
        esg = C.tile([128, 16], BF, tag="esg")
        for r in range(NCORES):
            QS[r % 3].dma_start(
                out=esg[:, 2 * r:2 * r + 2],
                in_=ag_out.ap()[AGP * r + 1024:AGP * r + 1152, 0:2])
        for kc in range(8):
            for r in range(NCORES):
                QS[(kc * NCORES + r) % 3].dma_start(
                    out=xall[:, 2048 * kc + TOK * r:2048 * kc + TOK * (r + 1)],
                    in_=ag_out.ap()[AGP * r + 128 * kc:
                                    AGP * r + 128 * (kc + 1), :])
        v.tensor_copy(es32[:], esg[:])

        # ============ Phase 2: QK + V ============
        with tc.tile_pool(name="ps_qk", bufs=2, space="PSUM") as PSQ, \
             tc.tile_pool(name="ps_v", bufs=3, space="PSUM") as PSV:
            for w in range(4):
                for of in range(2):
                    ps = PSQ.tile([128, 512], F32, tag="psqk")
                    for kc in range(8):
                        te.matmul(
                            ps[:],
                            wqk_s[:, (of * 8 + kc) * 128:(of * 8 + kc + 1) * 128],
                            xall[:, 2048 * kc + 512 * w:2048 * kc + 512 * (w + 1)],
                            start=(kc == 0), stop=(kc == 7))
                    v.tensor_scalar(
                        qkT[:, 2048 * of + 512 * w:2048 * of + 512 * (w + 1)],
                        ps[:], sm["b_qk"][:, of:of + 1], None, Alu.add)
                for tch in range(4 * w, 4 * w + 4):
                    psv = PSV.tile([128, 136], F32, tag="psv")
                    for kc in range(8):
                        te.matmul(
                            psv[:],
                            xall[:, 2048 * kc + 128 * tch:2048 * kc + 128 * (tch + 1)],
                            wv_s[:, 136 * kc:136 * (kc + 1)],
                            start=(kc == 0), stop=(kc == 7))
                    vt = vaug[:, 136 * tch:136 * (tch + 1)]
                    v.tensor_tensor(vt, psv[:], bvb[:], Alu.add)
                    v.tensor_scalar(vt, vt, es32[:, tch:tch + 1], None, Alu.mult)
                    for lh in range(HPC):
                        v.memset(vaug[:, 136 * tch + 68 * lh + 64:
                                      136 * tch + 68 * lh + 65], 1.0)

        # ============ Phase 3: attention ============
        att_stash = []
        with tc.tile_pool(name="att_sb", bufs=3) as ASB, \
             tc.tile_pool(name="ao_sb", bufs=4) as AOSB:
            with tc.tile_pool(name="ps_sc", bufs=2, space="PSUM") as PSS, \
                 tc.tile_pool(name="ps_ao", bufs=2, space="PSUM") as PSA:
                for G in range(4):
                    nkb = 4 * G + 4
                    ao0 = PSA.tile([65, 512], F32, tag="ao0")
                    ao1 = PSA.tile([65, 512], F32, tag="ao1")
                    for kb in range(nkb):
                        ps = PSS.tile([128, 1024], F32, tag="ps_sc")
                        te.matmul(ps[:, 0:512],
                                  qkT[0:64, 2048 + 128 * kb:2048 + 128 * (kb + 1)],
                                  qkT[0:64, 512 * G:512 * (G + 1)],
                                  start=True, stop=True)
                        te.matmul(ps[:, 512:1024],
                                  qkT[64:128, 2048 + 128 * kb:2048 + 128 * (kb + 1)],
                                  qkT[64:128, 512 * G:512 * (G + 1)],
                                  start=True, stop=True)
                        ex = ASB.tile([128, 1024], BF, tag="ex")
                        s.activation(ex[:], ps[:], Act.Exp)
                        j = kb - 4 * G
                        if 0 <= j < 4:
                            v.tensor_tensor(ex[:], ex[:],
                                            tri2_s[:, 1024 * j:1024 * (j + 1)],
                                            Alu.mult)
                        te.matmul(ao0[:], vaug[:, 136 * kb:136 * kb + 65],
                                  ex[:, 0:512],
                                  start=(kb == 0), stop=(kb == nkb - 1))
                        te.matmul(ao1[:], vaug[:, 136 * kb + 68:136 * kb + 133],
                                  ex[:, 512:1024],
                                  start=(kb == 0), stop=(kb == nkb - 1))
                    for h, ao in ((0, ao0), (1, ao1)):
                        aos = AOSB.tile([65, 512], F32, tag=f"aos{h}")
                        v.tensor_copy(aos[:], ao[0:65, :])
                        dent = denpA if h == 0 else denpB
                        v.tensor_copy(dent[32 * G:32 * G + 1, :],
                                      aos[64:65, :])
                        att_stash.append((h, G, aos))
            v.reciprocal(denpA[:], denpA[:])
            v.reciprocal(denpB[:], denpB[:])
            with tc.tile_pool(name="ps_rb", bufs=2, space="PSUM") as PSRB:
                for h, G, aos in att_stash:
                    dent = denpA if h == 0 else denpB
                    rrow = ASB.tile([1, 512], BF, tag="rrow")
                    v.tensor_copy(rrow[0:1, :], dent[32 * G:32 * G + 1, :])
                    rbp = PSRB.tile([64, 512], F32, tag="rbp")
                    te.matmul(rbp[:], onesr[:], rrow[:], start=True, stop=True)
                    v.tensor_tensor(
                        aosc[64 * h:64 * (h + 1), 512 * G:512 * (G + 1)],
                        aos[0:64, :], rbp[:], Alu.mult)

        # ============ Phase 4: AllToAll ============
        for r in range(NCORES):
            QS[r % 3].dma_start(out=a2a_in.ap()[128 * r:128 * (r + 1), :],
                                in_=aosc[:, TOK * r:TOK * (r + 1)])
        g.collective_compute("AllToAll", Alu.bypass, replica_groups=RG,
                             ins=[a2a_in.ap()], outs=[a2a_out.ap()])
        for r in range(NCORES):
            QS[r % 3].dma_start(out=aofull[:, TOK * r:TOK * (r + 1)],
                                in_=a2a_out.ap()[128 * r:128 * (r + 1), :])

        # ============ Phase 5: out proj + norm1(+ffconst) + norm2 ============
        with tc.tile_pool(name="ps_out", bufs=3, space="PSUM") as PSO, \
             tc.tile_pool(name="ps_r2", bufs=1, space="PSUM") as PSR2, \
             tc.tile_pool(name="tmp2", bufs=2) as TMP2:
            for of in range(8):
                ps = PSO.tile([128, TOK], F32, tag="ps_out")
                for kc in range(8):
                    te.matmul(
                        ps[:],
                        wout_s[:, (of * 8 + kc) * 128:(of * 8 + kc + 1) * 128],
                        aofull[:, TOK * kc:TOK * (kc + 1)],
                        start=(kc == 0), stop=(kc == 7))
                v.scalar_tensor_tensor(xt[:, TOK * of:TOK * (of + 1)],
                                       ps[:], sm["b_out"][:, of:of + 1],
                                       xt[:, TOK * of:TOK * (of + 1)],
                                       Alu.add, Alu.add)
            _ln_full(nc, tc, TMP2, PSR2, rows, xt, x1f, ones32,
                     sm["n1w"], sm["n1b"], epsap, 4)
            _ln_full(nc, tc, TMP2, PSR2, rows, x1f, outt, ones32,
                     sm["n2w"], sm["n2b"], epsap, 8,
                     out_dma=(t_out, QS))


# ----------------------------------------------------------------------------
# Full fallback path (original kernel)
# ----------------------------------------------------------------------------

def _prepare_full(inputs):
    f = lambda k: np.asarray(inputs[k], np.float32)
    x = f("x").reshape(S, D)
    qkv_w, qkv_b = f("qkv_w"), f("qkv_b")
    out_w, out_b = f("out_w") * 0.1, f("out_b") * 0.1
    ff1_w, ff1_b = f("ff1_w"), f("ff1_b")
    ff2_w, ff2_b = f("ff2_w"), f("ff2_b")
    ep1_w, ep1_b = f("ep1_w"), f("ep1_b")
    ep2_w, ep2_b = f("ep2_w"), f("ep2_b")
    ent_w, ent_b = f("ent_w"), f("ent_b")

    temp = (1.0 / np.sqrt(np.float32(HD))) / 0.1   # 1.25
    wq = qkv_w[0:D] * temp
    wk = qkv_w[D:2 * D]
    wv = qkv_w[2 * D:3 * D]
    bq = qkv_b[0:D] * temp
    bk = qkv_b[D:2 * D]
    bv = qkv_b[2 * D:3 * D]

    coeffs, bknot, fit_err = _fit_spline(f("knots"), f("spl_w"))

    def _make_tri_masks():
        out = np.zeros((128, 4 * 512), np.float32)
        for j in range(4):
            kk = np.arange(128)[:, None] + 128 * j
            q = np.arange(512)[None, :]
            out[:, 512 * j:512 * (j + 1)] = (kk <= q).astype(np.float32)
        return out

    shared = {
        "tri": _make_tri_masks().astype(NPBF),
        "ones32": np.ones((128, 1), np.float32),
        "onesb": np.ones((128, 1), NPBF),
        "wff1": _pack_lhsT(ff1_w.T, 32, 8).astype(NPBF),
        "wff2": _pack_lhsT(ff2_w.T, 8, 32).astype(NPBF),
        "wep1": _pack_lhsT(ep1_w.T, 2, 32).astype(NPBF),
        "wout": _pack_lhsT(out_w.T, 8, 8).astype(NPBF),
        "wep2": np.ascontiguousarray(
            ep2_w.reshape(2, 128).T).astype(NPBF),          # [128, 2]
        "went": np.ascontiguousarray(
            ent_w.reshape(8, 128).T).astype(NPBF),          # [128, 8]
        "b_ff1": _col_pack(ff1_b, 32),
        "b_ff2": _col_pack(ff2_b, 8),
        "b_ep1": _col_pack(ep1_b, 2),
        "b_out": _col_pack(out_b, 8),
        "lnw": _col_pack(f("ln_attn_w"), 8),
        "lnb": _col_pack(f("ln_attn_b"), 8),
        "n1w": _col_pack(f("norm1_w"), 8),
        "n1b": _col_pack(f("norm1_b"), 8),
        "n2w": _col_pack(f("norm2_w"), 8),
        "n2b": _col_pack(f("norm2_b"), 8),
        "eplw": _col_pack(f("ep_ln_w"), 2),
        "eplb": _col_pack(f("ep_ln_b"), 2),
    }

    scalars = {
        "ent_b": float(ent_b.reshape(-1)[0]),
        "ep2_b": float(ep2_b.reshape(-1)[0]),
        "coeffs": coeffs,
        "bknot": bknot,
        "fit_err": fit_err,
        "fast": False,
    }

    in_maps = []
    for c in range(NCORES):
        m = dict(shared)
        xc = x[c * TOK:(c + 1) * TOK]                        # [256, D]
        xT = np.ascontiguousarray(xc.T)                      # [D, 256]
        m["xT"] = np.ascontiguousarray(
            xT.reshape(8, 128, TOK).transpose(1, 0, 2).reshape(128, 8 * TOK))
        h0 = c * HPC
        wq_c = wq[h0 * HD:(h0 + HPC) * HD]                   # [128, D]
        wk_c = wk[h0 * HD:(h0 + HPC) * HD]
        wqk_t = np.concatenate([wq_c, wk_c], 0).T            # [D, 256]
        m["wqk"] = _pack_lhsT(wqk_t, 2, 8).astype(NPBF)
        m["b_qk"] = np.ascontiguousarray(np.stack(
            [bq[h0 * HD:(h0 + HPC) * HD],
             bk[h0 * HD:(h0 + HPC) * HD]], -1).astype(np.float32))
        wv_c = wv[h0 * HD:(h0 + HPC) * HD].T                 # [D, 128]
        wva = np.zeros((D, 136), np.float32)
        bva = np.zeros((1, 136), np.float32)
        for lh in range(HPC):
            wva[:, 68 * lh:68 * lh + 64] = wv_c[:, 64 * lh:64 * lh + 64]
            bva[0, 68 * lh:68 * lh + 64] = \
                bv[(h0 + lh) * HD:(h0 + lh + 1) * HD]
        m["wv"] = np.ascontiguousarray(
            wva.reshape(8, 128, 136).transpose(1, 0, 2).reshape(128, 8 * 136)
        ).astype(NPBF)
        m["b_v"] = bva
        in_maps.append(m)

    return in_maps, scalars


# The original full-path program builder (verbatim from the v1 kernel).
def _build_program_full(sc):
    nc = bacc.Bacc("TRN2", target_bir_lowering=False, debug=False,
                   num_devices=NCORES)

    def din(name, shape, dtype):
        return nc.dram_tensor(name, list(shape), dtype, kind="ExternalInput")

    tin = {
        "xT": din("xT", (128, 8 * TOK), F32),
        "wqk": din("wqk", (128, 2048), BF),
        "wv": din("wv", (128, 8 * 136), BF),
        "went": din("went", (128, 8), BF),
        "wout": din("wout", (128, 8192), BF),
        "wff1": din("wff1", (128, 32768), BF),
        "wff2": din("wff2", (128, 32768), BF),
        "wep1": din("wep1", (128, 8192), BF),
        "wep2": din("wep2", (128, 2), BF),
        "tri": din("tri", (128, 2048), BF),
        "ones32": din("ones32", (128, 1), F32),
        "onesb": din("onesb", (128, 1), BF),
        "b_qk": din("b_qk", (128, 2), F32),
        "b_v": din("b_v", (1, 136), F32),
        "b_out": din("b_out", (128, 8), F32),
        "b_ff1": din("b_ff1", (128, 32), F32),
        "b_ff2": din("b_ff2", (128, 8), F32),
        "b_ep1": din("b_ep1", (128, 2), F32),
        "lnw": din("lnw", (128, 8), F32),
        "lnb": din("lnb", (128, 8), F32),
        "n1w": din("n1w", (128, 8), F32),
        "n1b": din("n1b", (128, 8), F32),
        "n2w": din("n2w", (128, 8), F32),
        "n2b": din("n2b", (128, 8), F32),
        "eplw": din("eplw", (128, 2), F32),
        "eplb": din("eplb", (128, 2), F32),
    }
    t_out = nc.dram_tensor("out", [128, 8 * TOK], F32, kind="ExternalOutput")
    ag_in = nc.dram_tensor("ag_in", [1024, TOK], BF, kind="Internal")
    ag_out = nc.dram_tensor("ag_out", [8192, TOK], BF, kind="Internal",
                            addr_space="Shared")
    a2a_in = nc.dram_tensor("a2a_in", [1024, TOK], BF, kind="Internal")
    a2a_out = nc.dram_tensor("a2a_out", [1024, TOK], BF, kind="Internal")

    with tile.TileContext(nc) as tc:
        _emit_full(nc, tc, tin, t_out, ag_in, ag_out, a2a_in, a2a_out, sc)
    nc.compile()
    return nc


def _emit_full(nc, tc, tin, t_out, ag_in, ag_out, a2a_in, a2a_out, sc):
    v = nc.vector
    s = nc.scalar
    g = nc.gpsimd
    te = nc.tensor
    dma = nc.sync.dma_start
    c0, c1, c2, c3a, c4a, c5, c6, c7 = sc["coeffs"]
    RG = [list(range(NCORES))]

    with tc.tile_pool(name="persist", bufs=1) as P, \
         tc.tile_pool(name="consts", bufs=1) as C, \
         tc.tile_pool(name="rows", bufs=1) as R:

        onesr = P.tile([1, 64], BF, tag="onesr")
        xt = P.tile([128, 8 * TOK], F32, tag="xt")
        qkT = P.tile([128, 4096], BF, tag="qkT")
        vaug = P.tile([128, 16 * 136], BF, tag="vaug")
        es = P.tile([128, 16], F32, tag="es")
        aosc = P.tile([128, 2048], BF, tag="aosc")
        aofull = P.tile([128, 8 * TOK], BF, tag="aofull")
        x1f = P.tile([128, 8 * TOK], F32, tag="x1f")
        x1b = P.tile([128, 8 * TOK], BF, tag="x1b")
        actt = P.tile([128, 8192], BF, tag="actt")
        outt = P.tile([128, 8 * TOK], F32, tag="outt")

        sm = {}
        for nm, t in tin.items():
            if nm in ("xT", "wff1", "wff2", "wep1", "wout"):
                continue
            sm[nm] = C.tile(list(t.shape), t.dtype, tag=nm, name="sm_" + nm)
            dma(out=sm[nm][:], in_=t.ap())
        ones32, onesb, tri = sm["ones32"], sm["onesb"], sm["tri"]
        bvb = C.tile([128, 136], F32, tag="bvb")
        g.partition_broadcast(bvb[:], sm["b_v"][0:1, :])
        cst = C.tile([128, 5], F32, tag="cst")
        v.memset(cst[:, 0:1], EPS)
        v.memset(cst[:, 1:2], -sc["ent_b"])
        v.memset(cst[:, 2:3], -sc["ep2_b"])
        v.memset(cst[:, 3:4], -sc["bknot"])
        v.memset(cst[:, 4:5], sc["bknot"])

        v.memset(onesr[:], 1.0)
        rows = R.tile([1, 24 * TOK], F32, tag="rows")
        rsf = lambda k: rows[0:1, k * TOK:(k + 1) * TOK]
        denpA = R.tile([128, 512], F32, tag="denpA")
        denpB = R.tile([128, 512], F32, tag="denpB")

        dma(out=xt[:], in_=tin["xT"].ap())

        XA_cm = tc.tile_pool(name="xa_pool", bufs=1)
        XA = XA_cm.__enter__()
        xall = XA.tile([128, 16384], BF, tag="xall")
        with tc.tile_pool(name="ps_r1", bufs=1, space="PSUM") as PSR, \
             tc.tile_pool(name="tmp1", bufs=2) as TMP:
            t_sx = PSR.tile([1, TOK], F32, tag="sx1p")
            t_sx2 = PSR.tile([1, TOK], F32, tag="sx2p")
            sx, sx2 = t_sx[:], t_sx2[:]
            for kc in range(8):
                te.matmul(sx, ones32[:], xt[:, TOK * kc:TOK * (kc + 1)],
                          start=(kc == 0), stop=(kc == 7))
            xsq = TMP.tile([128, TOK], F32, tag="xsq")
            for kc in range(8):
                v.tensor_tensor(xsq[:], xt[:, TOK * kc:TOK * (kc + 1)],
                                xt[:, TOK * kc:TOK * (kc + 1)], Alu.mult)
                te.matmul(sx2, ones32[:], xsq[:],
                          start=(kc == 0), stop=(kc == 7))
            _ln_rows(nc, rsf(0), rsf(1), rsf(2), sx, sx2, D, cst[0:1, 0:1])
            mu_b = TMP.tile([128, TOK], F32, tag="mu_b")
            s_b = TMP.tile([128, TOK], F32, tag="s_b")
            g.partition_broadcast(mu_b[:], rsf(0))
            g.partition_broadcast(s_b[:], rsf(1))
            tm = TMP.tile([128, TOK], F32, tag="tm")
            for kc in range(8):
                xlc = TMP.tile([128, TOK], BF, tag="xlc")
                v.tensor_tensor(tm[:], xt[:, TOK * kc:TOK * (kc + 1)],
                                mu_b[:], Alu.subtract)
                v.tensor_tensor(tm[:], tm[:], s_b[:], Alu.mult)
                v.tensor_scalar(xlc[:], tm[:],
                                sm["lnw"][:, kc:kc + 1], sm["lnb"][:, kc:kc + 1],
                                Alu.mult, Alu.add)
                dma(out=ag_in.ap()[128 * kc:128 * (kc + 1), :], in_=xlc[:])
        g.collective_compute("AllGather", Alu.bypass, replica_groups=RG,
                             ins=[ag_in.ap()], outs=[ag_out.ap()])
        for kc in range(8):
            for r in range(NCORES):
                dma(out=xall[:, 2048 * kc + TOK * r:2048 * kc + TOK * (r + 1)],
                    in_=ag_out.ap()[1024 * r + 128 * kc:
                                    1024 * r + 128 * (kc + 1), :])

        with tc.tile_pool(name="wq_pool", bufs=1) as WQ, \
             tc.tile_pool(name="ps_qk", bufs=2, space="PSUM") as PSQ, \
             tc.tile_pool(name="ps_ev", bufs=3, space="PSUM") as PSV:
            went_s = WQ.tile([128, 8], BF, tag="went_s")
            dma(out=went_s[:], in_=tin["went"].ap())
            pse = PSV.tile([128, 16], F32, tag="pse", bufs=1)
            for tch in range(16):
                for kc in range(8):
                    te.matmul(
                        pse[:, tch:tch + 1],
                        xall[:, 2048 * kc + 128 * tch:2048 * kc + 128 * (tch + 1)],
                        went_s[:, kc:kc + 1],
                        start=(kc == 0), stop=(kc == 7))
            est = WQ.tile([128, 16], F32, tag="est")
            s.activation(est[:], pse[:], Act.Exp, bias=cst[:, 1:2], scale=-1.0)
            v.tensor_scalar(est[:], est[:], 1.0, None, Alu.add)
            v.reciprocal(es[:], est[:])
            v.tensor_scalar(es[:], es[:], 0.1, 2.0, Alu.max, Alu.min)
            wv_s = WQ.tile([128, 8 * 136], BF, tag="wv_s")
            dma(out=wv_s[:], in_=tin["wv"].ap())
            for tch in range(16):
                psv = PSV.tile([128, 136], F32, tag="psv", bufs=2)
                for kc in range(8):
                    te.matmul(
                        psv[:],
                        xall[:, 2048 * kc + 128 * tch:2048 * kc + 128 * (tch + 1)],
                        wv_s[:, 136 * kc:136 * (kc + 1)],
                        start=(kc == 0), stop=(kc == 7))
                vt = vaug[:, 136 * tch:136 * (tch + 1)]
                v.tensor_tensor(vt, psv[:], bvb[:], Alu.add)
                v.tensor_scalar(vt, vt, es[:, tch:tch + 1], None, Alu.mult)
                for lh in range(HPC):
                    v.memset(vaug[:, 136 * tch + 68 * lh + 64:
                                  136 * tch + 68 * lh + 65], 1.0)

            wqk_s = WQ.tile([128, 2048], BF, tag="wqk_s")
            dma(out=wqk_s[:], in_=tin["wqk"].ap())
            for of in range(2):
                for w in range(4):
                    ps = PSQ.tile([128, 512], F32, tag="psqk")
                    for kc in range(8):
                        te.matmul(
                            ps[:],
                            wqk_s[:, (of * 8 + kc) * 128:(of * 8 + kc + 1) * 128],
                            xall[:, 2048 * kc + 512 * w:2048 * kc + 512 * (w + 1)],
                            start=(kc == 0), stop=(kc == 7))
                    v.tensor_scalar(
                        qkT[:, 2048 * of + 512 * w:2048 * of + 512 * (w + 1)],
                        ps[:], sm["b_qk"][:, of:of + 1], None, Alu.add)

        XA_cm.__exit__(None, None, None)

        att_stash = []
        with tc.tile_pool(name="ps_sc", bufs=2, space="PSUM") as PSS, \
             tc.tile_pool(name="ps_ao", bufs=2, space="PSUM") as PSA, \
             tc.tile_pool(name="att_sb", bufs=3) as ASB, \
             tc.tile_pool(name="ao_sb", bufs=8) as AOSB:
            for lh in range(HPC):
                hq = qkT[64 * lh:64 * (lh + 1), 0:2048]
                hk = qkT[64 * lh:64 * (lh + 1), 2048:4096]
                for G in range(4):
                    nkb = 4 * G + 4
                    ao = PSA.tile([65, 512], F32, tag="ao")
                    for pj in range(nkb // 2):
                        ps = PSS.tile([128, 1024], F32, tag="ps_sc")
                        ex = ASB.tile([128, 1024], BF, tag="ex")
                        for half in range(2):
                            kb = 2 * pj + half
                            te.matmul(ps[:, 512 * half:512 * (half + 1)],
                                      hk[:, 128 * kb:128 * (kb + 1)],
                                      hq[:, 512 * G:512 * (G + 1)],
                                      start=True, stop=True)
                        s.activation(ex[:], ps[:], Act.Exp)
                        for half in range(2):
                            kb = 2 * pj + half
                            j = kb - 4 * G
                            exh = ex[:, 512 * half:512 * (half + 1)]
                            if 0 <= j < 4:
                                v.tensor_tensor(
                                    exh, exh, tri[:, 512 * j:512 * (j + 1)],
                                    Alu.mult)
                            te.matmul(
                                ao[:],
                                vaug[:, 136 * kb + 68 * lh:
                                     136 * kb + 68 * lh + 65],
                                exh,
                                start=(kb == 0), stop=(kb == nkb - 1))
                    aos = AOSB.tile([65, 512], F32, tag="aos")
                    s.copy(aos[:], ao[0:65, :])
                    dent = denpA if lh == 0 else denpB
                    v.tensor_copy(dent[32 * G:32 * G + 1, :], aos[64:65, :])
                    att_stash.append((lh, G, aos))
            v.reciprocal(denpA[:], denpA[:])
            v.reciprocal(denpB[:], denpB[:])
            for lh, G, aos in att_stash:
                rrow = ASB.tile([1, 512], BF, tag="rrow")
                dent = denpA if lh == 0 else denpB
                v.tensor_copy(rrow[0:1, :], dent[32 * G:32 * G + 1, :])
                rbp = PSA.tile([64, 512], F32, tag="rbp")
                te.matmul(rbp[:], onesr[:], rrow[:], start=True, stop=True)
                v.tensor_tensor(
                    aosc[64 * lh:64 * (lh + 1), 512 * G:512 * (G + 1)],
                    aos[0:64, :], rbp[:], Alu.mult)

        for r in range(NCORES):
            dma(out=a2a_in.ap()[128 * r:128 * (r + 1), :],
                in_=aosc[:, TOK * r:TOK * (r + 1)])
        g.collective_compute("AllToAll", Alu.bypass, replica_groups=RG,
                             ins=[a2a_in.ap()], outs=[a2a_out.ap()])
        for r in range(NCORES):
            dma(out=aofull[:, TOK * r:TOK * (r + 1)],
                in_=a2a_out.ap()[128 * r:128 * (r + 1), :])

        with tc.tile_pool(name="wo_pool", bufs=1) as WO, \
             tc.tile_pool(name="ps_out", bufs=3, space="PSUM") as PSO, \
             tc.tile_pool(name="ps_r2", bufs=1, space="PSUM") as PSR2, \
             tc.tile_pool(name="tmp2", bufs=2) as TMP2:
            wout_s = WO.tile([128, 8192], BF, tag="wout_s")
            for qq in range(4):
                [nc.sync, nc.gpsimd, nc.sync, nc.gpsimd][qq].dma_start(
                    out=wout_s[:, 2048 * qq:2048 * (qq + 1)],
                    in_=tin["wout"].ap()[:, 2048 * qq:2048 * (qq + 1)])
            for of in range(8):
                ps = PSO.tile([128, TOK], F32, tag="ps_out")
                for kc in range(8):
                    te.matmul(
                        ps[:],
                        wout_s[:, (of * 8 + kc) * 128:(of * 8 + kc + 1) * 128],
                        aofull[:, TOK * kc:TOK * (kc + 1)],
                        start=(kc == 0), stop=(kc == 7))
                v.scalar_tensor_tensor(xt[:, TOK * of:TOK * (of + 1)],
                                       ps[:], sm["b_out"][:, of:of + 1],
                                       xt[:, TOK * of:TOK * (of + 1)],
                                       Alu.add, Alu.add)
            _ln_full_v1(nc, tc, TMP2, PSR2, rows, xt, x1f, x1b, ones32,
                        sm["n1w"], sm["n1b"], cst[0:1, 0:1])

        with tc.tile_pool(name="w1_pool", bufs=3) as W1, \
             tc.tile_pool(name="ps_h", bufs=2, space="PSUM") as PSH, \
             tc.tile_pool(name="ps_r3", bufs=1, space="PSUM") as PSR3, \
             tc.tile_pool(name="tmp3", bufs=1) as TMP3:
            hb = TMP3.tile([128, 8192], BF, tag="hb")
            t_sh = PSR3.tile([1, TOK], F32, tag="shp")
            t_sh2 = PSR3.tile([1, TOK], F32, tag="sh2p")
            t_se1 = PSR3.tile([1, TOK], F32, tag="se1p")
            t_se2 = PSR3.tile([1, TOK], F32, tag="se2p")
            t_pse2 = PSR3.tile([1, TOK], F32, tag="pse2p")
            sh, sh2, se1, se2, pse2 = (t_sh[:], t_sh2[:], t_se1[:],
                                       t_se2[:], t_pse2[:])
            hsqp = TMP3.tile([128, TOK], BF, tag="hsqp")
            for c in range(32):
                w1t = W1.tile([128, 1024], BF, tag="w1t", bufs=6)
                dmae = [nc.sync, nc.gpsimd][c % 2].dma_start
                dmae(out=w1t[:], in_=tin["wff1"].ap()[:, 1024 * c:1024 * (c + 1)])
                ps = PSH.tile([128, TOK], F32, tag="ps_h")
                for kc in range(8):
                    te.matmul(ps[:], w1t[:, 128 * kc:128 * (kc + 1)],
                              x1b[:, TOK * kc:TOK * (kc + 1)],
                              start=(kc == 0), stop=(kc == 7))
                hs = hb[:, TOK * c:TOK * (c + 1)]
                s.activation(hs, ps[:], Act.Identity,
                             bias=sm["b_ff1"][:, c:c + 1])
                g.tensor_tensor(hsqp[:], hs, hs, Alu.mult)
                te.matmul(sh, onesb[:], hs, start=(c == 0), stop=(c == 31))
                te.matmul(sh2, onesb[:], hsqp[:], start=(c == 0), stop=(c == 31))
            wep1_s = W1.tile([128, 8192], BF, tag="wep1_s", bufs=1)
            for qq in range(4):
                [nc.sync, nc.gpsimd, nc.sync, nc.gpsimd][qq].dma_start(
                    out=wep1_s[:, 2048 * qq:2048 * (qq + 1)],
                    in_=tin["wep1"].ap()[:, 2048 * qq:2048 * (qq + 1)])
            epb = TMP3.tile([128, 2 * TOK], BF, tag="epb")
            epsq = TMP3.tile([128, TOK], BF, tag="epsq")
            for of in range(2):
                ps = PSH.tile([128, TOK], F32, tag="ps_h")
                for kc in range(32):
                    te.matmul(
                        ps[:],
                        wep1_s[:, (of * 32 + kc) * 128:(of * 32 + kc + 1) * 128],
                        hb[:, TOK * kc:TOK * (kc + 1)],
                        start=(kc == 0), stop=(kc == 31))
                s.activation(epb[:, TOK * of:TOK * (of + 1)], ps[:],
                             Act.Identity, bias=sm["b_ep1"][:, of:of + 1])
                v.tensor_tensor(epsq[:], epb[:, TOK * of:TOK * (of + 1)],
                                epb[:, TOK * of:TOK * (of + 1)], Alu.mult)
                te.matmul(se1, onesb[:], epb[:, TOK * of:TOK * (of + 1)],
                          start=(of == 0), stop=(of == 1))
                te.matmul(se2, onesb[:], epsq[:],
                          start=(of == 0), stop=(of == 1))
            _ln_rows(nc, rsf(3), rsf(4), rsf(5), se1, se2, D16, cst[0:1, 0:1])
            mue_b = TMP3.tile([128, TOK], F32, tag="mue_b")
            see_b = TMP3.tile([128, TOK], F32, tag="see_b")
            g.partition_broadcast(mue_b[:], rsf(3))
            g.partition_broadcast(see_b[:], rsf(4))
            relub = TMP3.tile([128, 2 * TOK], BF, tag="relub")
            tm3 = TMP3.tile([128, TOK], F32, tag="tm3")
            for of in range(2):
                v.tensor_tensor(tm3[:], epb[:, TOK * of:TOK * (of + 1)],
                                mue_b[:], Alu.subtract)
                v.tensor_tensor(tm3[:], tm3[:], see_b[:], Alu.mult)
                v.tensor_scalar(tm3[:], tm3[:], sm["eplw"][:, of:of + 1],
                                sm["eplb"][:, of:of + 1], Alu.mult, Alu.add)
                v.tensor_scalar(relub[:, TOK * of:TOK * (of + 1)], tm3[:],
                                0.0, None, Alu.max)
            for of in range(2):
                te.matmul(pse2, sm["wep2"][:, of:of + 1],
                          relub[:, TOK * of:TOK * (of + 1)],
                          start=(of == 0), stop=(of == 1))
            erow = rsf(6)
            s.activation(erow, pse2, Act.Exp, bias=cst[0:1, 2:3], scale=-1.0)
            v.tensor_scalar(erow, erow, 1.0, None, Alu.add)
            v.reciprocal(erow, erow)
            v.tensor_scalar(erow, erow, 0.1, 1.0, Alu.mult, Alu.add)

            _spline_rows(nc, rsf, sh, sh2, cst[0:1, 0:1])

            muh_b = TMP3.tile([128, TOK], F32, tag="muh_b")
            Sh_b = TMP3.tile([128, TOK], F32, tag="Sh_b")
            em_b = TMP3.tile([128, TOK], F32, tag="em_b")
            g.partition_broadcast(muh_b[:], rsf(7))
            g.partition_broadcast(Sh_b[:], rsf(8))
            g.partition_broadcast(em_b[:], rsf(6))
            murep = TMP3.tile([128, 2048], BF, tag="murep")
            Srep = TMP3.tile([128, 2048], BF, tag="Srep")
            emrep = TMP3.tile([128, 2048], BF, tag="emrep")
            for (src, dst) in ((muh_b, murep), (Sh_b, Srep), (em_b, emrep)):
                v.tensor_copy(dst[:], src[:].unsqueeze(1).to_broadcast((128, 8, TOK)))

            with tc.tile_pool(name="spl", bufs=1) as SPL:
                for gi in range(4):
                    hbs = hb[:, 2048 * gi:2048 * (gi + 1)]
                    u = SPL.tile([128, 2048], BF, tag="u")
                    acc = SPL.tile([128, 2048], BF, tag="acc")
                    t1 = SPL.tile([128, 2048], BF, tag="t1")
                    t2 = SPL.tile([128, 2048], BF, tag="t2")
                    t3 = SPL.tile([128, 2048], BF, tag="t3")
                    v.tensor_tensor(u[:], hbs, murep[:], Alu.subtract)
                    v.tensor_tensor(u[:], u[:], Srep[:], Alu.mult)
                    v.tensor_scalar(u[:], u[:], -UDOM, UDOM, Alu.max, Alu.min)
                    v.tensor_tensor(t1[:], u[:], u[:], Alu.mult)
                    s.activation(t3[:], u[:], Act.Abs)
                    v.tensor_scalar(acc[:], t1[:], c2, c0, Alu.mult, Alu.add)
                    v.scalar_tensor_tensor(acc[:], u[:], c1, acc[:],
                                           Alu.mult, Alu.add)
                    v.scalar_tensor_tensor(acc[:], t3[:], c3a, acc[:],
                                           Alu.mult, Alu.add)
                    s.activation(t2[:], u[:], Act.Abs, bias=cst[:, 3:4])
                    v.scalar_tensor_tensor(acc[:], t2[:], c6, acc[:],
                                           Alu.mult, Alu.add)
                    s.activation(t2[:], u[:], Act.Abs, bias=cst[:, 4:5])
                    v.scalar_tensor_tensor(acc[:], t2[:], c7, acc[:],
                                           Alu.mult, Alu.add)
                    v.tensor_tensor(acc[:], acc[:], emrep[:], Alu.mult)
                    v.tensor_scalar(actt[:, 2048 * gi:2048 * (gi + 1)],
                                    acc[:], 1.0, -1.0, Alu.min, Alu.max)

        with tc.tile_pool(name="w2_pool", bufs=3) as W2, \
             tc.tile_pool(name="ps_f2", bufs=3, space="PSUM") as PSF, \
             tc.tile_pool(name="ps_r4", bufs=1, space="PSUM") as PSR4, \
             tc.tile_pool(name="tmp4", bufs=2) as TMP4:
            r2 = TMP4.tile([128, 8 * TOK], F32, tag="r2")
            for of in range(8):
                w2t = W2.tile([128, 4096], BF, tag="w2t", bufs=4)
                dmae = [nc.sync, nc.gpsimd][of % 2].dma_start
                dmae(out=w2t[:], in_=tin["wff2"].ap()[:, 4096 * of:4096 * (of + 1)])
                ps = PSF.tile([128, TOK], F32, tag="ps_f2")
                for kc in range(32):
                    te.matmul(ps[:], w2t[:, 128 * kc:128 * (kc + 1)],
                              actt[:, TOK * kc:TOK * (kc + 1)],
                              start=(kc == 0), stop=(kc == 31))
                v.scalar_tensor_tensor(r2[:, TOK * of:TOK * (of + 1)],
                                       ps[:], sm["b_ff2"][:, of:of + 1],
                                       x1f[:, TOK * of:TOK * (of + 1)],
                                       Alu.add, Alu.add)
            _ln_full_v1(nc, tc, TMP4, PSR4, rows, r2, outt, None, ones32,
                        sm["n2w"], sm["n2b"], cst[0:1, 0:1])
        dma(out=t_out.ap(), in_=outt[:])


def _spline_rows(nc, rsf, sh, sh2, epsap):
    v, s = nc.vector, nc.scalar
    mu = rsf(7)
    S_ = rsf(8)
    var = rsf(13)
    t1 = rsf(14)
    t2 = rsf(15)
    v.tensor_scalar(mu, sh, 1.0 / FF, None, Alu.mult)
    v.tensor_tensor(var, mu, mu, Alu.mult)
    v.tensor_scalar(t1, sh2, 1.0 / FF, None, Alu.mult)
    v.tensor_tensor(var, t1, var, Alu.subtract)
    s.activation(t1, var, Act.Ln, bias=epsap)
    s.activation(t1, t1, Act.Exp, scale=0.5)
    v.tensor_scalar(t2, var, EPS, None, Alu.add)
    v.reciprocal(t2, t2)
    v.tensor_tensor(t2, t2, var, Alu.mult)
    v.tensor_scalar(t2, t2, float(FF), None, Alu.mult)
    s.activation(t2, t2, Act.Ln, bias=epsap)
    s.activation(t2, t2, Act.Exp, scale=0.5)
    v.tensor_scalar(t2, t2, 1.0, None, Alu.add)
    v.tensor_tensor(t2, t2, t1, Alu.mult)
    v.reciprocal(S_, t2)


def _ln_full_v1(nc, tc, TMP, PSR, rows, src, dstf, dstb, ones32, wcol, bcol,
                epsap):
    v, s, g, te = nc.vector, nc.scalar, nc.gpsimd, nc.tensor
    T = TOK
    rsf = lambda k: rows[0:1, k * T:(k + 1) * T]
    t_sx = PSR.tile([1, T], F32, tag="lnsxp")
    t_sx2 = PSR.tile([1, T], F32, tag="lnsx2p")
    sx, sx2 = t_sx[:], t_sx2[:]
    for kc in range(8):
        te.matmul(sx, ones32[:], src[:, T * kc:T * (kc + 1)],
                  start=(kc == 0), stop=(kc == 7))
    xsq = TMP.tile([128, T], F32, tag="lnxsq")
    for kc in range(8):
        v.tensor_tensor(xsq[:], src[:, T * kc:T * (kc + 1)],
                        src[:, T * kc:T * (kc + 1)], Alu.mult)
        te.matmul(sx2, ones32[:], xsq[:], start=(kc == 0), stop=(kc == 7))
    _ln_rows(nc, rsf(9), rsf(10), rsf(11), sx, sx2, D, epsap)
    mu_b = TMP.tile([128, T], F32, tag="lnmu_b")
    s_b = TMP.tile([128, T], F32, tag="lns_b")
    g.partition_broadcast(mu_b[:], rsf(9))
    g.partition_broadcast(s_b[:], rsf(10))
    tm = TMP.tile([128, T], F32, tag="lntm")
    for kc in range(8):
        v.tensor_tensor(tm[:], src[:, T * kc:T * (kc + 1)], mu_b[:],
                        Alu.subtract)
        v.tensor_tensor(tm[:], tm[:], s_b[:], Alu.mult)
        v.tensor_scalar(dstf[:, T * kc:T * (kc + 1)], tm[:],
                        wcol[:, kc:kc + 1], bcol[:, kc:kc + 1],
                        Alu.mult, Alu.add)
        if dstb is not None:
            v.tensor_copy(dstb[:, T * kc:T * (kc + 1)],
                          dstf[:, T * kc:T * (kc + 1)])


# ----------------------------------------------------------------------------
# Entry point
# ----------------------------------------------------------------------------

def _prepare(inputs):
    if _const_act_ok(np.asarray(inputs["knots"], np.float32),
                     np.asarray(inputs["spl_w"], np.float32)):
        return _prepare_fast(inputs)
    return _prepare_full(inputs)


def kernel(**inputs):
    in_maps, sc = _prepare(inputs)
    if sc["fast"]:
        key = "fast:" + hashlib.sha256(
            repr(sc["ent_b"]).encode()).hexdigest()
        if key not in _prog_cache:
            _prog_cache[key] = _build_fast(sc)
    else:
        key = "full:" + hashlib.sha256(
            repr((sc["coeffs"], sc["bknot"], sc["ent_b"], sc["ep2_b"])).encode()
        ).hexdigest()
        if key not in _prog_cache:
            _prog_cache[key] = _build_program_full(sc)
    nc = _prog_cache[key]
    res = bass_utils.run_bass_kernel_spmd(nc, in_maps,
                                          core_ids=list(range(NCORES)))
    out = np.empty((1, S, D), np.float32)
    for c in range(NCORES):
        oc = np.asarray(res.results[c]["out"], np.float32)   # [128, 8*TOK]
        ot = oc.reshape(128, 8, TOK).transpose(1, 0, 2).reshape(D, TOK)
        out[0, c * TOK:(c + 1) * TOK, :] = ot.T
    return out
